# revision 25
# baseline (speedup 1.0000x reference)
"""Trainium2 Bass kernel for nn_CCM: per-pixel complex 3x3 conv mask.

Math (per batch element b, sharded 1 batch element per NeuronCore):
  y[t,f] = sum_{n=0..8} A_n[t,f] * X[t+i(n)-2, f+j(n)-1]   (complex)
with A_n = m_n + w * m_{9+n} + conj(w) * m_{18+n}, w = -1/2 + i*sqrt(3)/2:
  Ar_n = m_n - 0.5*(m_{9+n} + m_{18+n})
  Ai_n = s * (m_{9+n} - m_{18+n}),  s = sqrt(3)/2
X = xr + i*xi, zero padded (causal in t: 2 top; symmetric in f: 1,1).

v4 design (trace-driven, from v1-v3):
- The kernel is jointly limited by the 29.8MB fp32 load stream (~84us at
  358GB/s) and the DVE. Everything else is arranged to keep both saturated:
- Per-tap products accumulate into PSUM fp32 via identity-weight matmuls on
  the PE (removes all accumulation adds from the DVE; better precision than
  bf16 accumulators).
- The +-s basis scale lives in two extra SCALED x planes (xrs=+s*xr,
  xin=-s*xi); th=-0.5*(m9+m18) runs on ACT with one-tap lookahead so the
  DVE never waits. DVE per tap: t1, d, ar + 4 plain 2x tensor_tensor muls.
- x stages via HWDGE (sync) as fp32 so the SWDGE m-stream starts ~4us
  earlier; x transposes run in fp32, plane copies cast to bf16.
- PSUM holds 4096 fp32/partition but the output needs 2*2056 so the f=256
  column accumulates in SBUF via tiny strided DVE adds (j==2 taps are zero
  there and skipped); it stores via a direct scatter SWDGE cast DMA.
- Output: per (f-half, comp) ONE psum bank collects all 8 tau-row
  transposes (start=False accumulate-into-disjoint-elements), then ONE
  contiguous-source copy into the bf16 staging tile; casting SWDGE stores.
- Tap 8's m tiles load in tau-quarters so its products/dr ains/transposes
  pipeline with the last DMA arrivals.
- PSUM banks are time-shared via same-tag tile reuse: head x-transposes ->
  accumulators -> tail output collectors.
"""

import sys
import numpy as np

sys.path.insert(0, "/opt/trn_rl_repo")

B = 8
C = 27
T = 1000
F = 257
TP = 125          # partitions
TAU = 8           # t = 8*p + tau
NS = 10           # slots in x planes: tau in [-2, 8)
SROW = 260        # x plane slot row width (elements)
MW = TAU * F      # 2056: m / prod tile width (flat, rows of 257)
AW = TAU * 256    # 2048: psum accumulator width (rows of 256)
PLW = NS * SROW   # 2600: x plane width
QW = 2 * F        # 514: one tau-quarter of a flat m plane
SQ3H = float(np.sqrt(3.0) / 2.0)

_CACHE = {}


def _emit(ctx, tc, m_ap, x_ap, id_ap, y_ap):
    import concourse.mybir as mybir

    nc = tc.nc
    f32 = mybir.dt.float32
    bf16 = mybir.dt.bfloat16
    FCS = [(0, 128), (128, 128), (256, 1)]   # f chunks for transposes
    SLOT_GROUPS = [(0, 4), (4, 4), (8, 2)]   # batches of slots per psum tile

    const = ctx.enter_context(tc.tile_pool(name="const", bufs=1))
    mcpool = ctx.enter_context(tc.tile_pool(name="mc", bufs=1))
    xpool = tc.alloc_tile_pool(name="xstage", bufs=1)
    # One PSUM pool; the 8 banks rotate roles via tag reuse:
    # head ptg transposes -> accr/acci accumulators -> tail out collectors.
    psum = ctx.enter_context(tc.tile_pool(name="psum", bufs=1, space="PSUM"))
    BTAGS = [f"bank{i}" for i in range(8)]

    # ---- SWDGE casting-load order: tap 0's pair first (it gates the DVE's
    # first op), then identb (gates PE transposes), tap 0's single, then x,
    # then the remaining taps. ident f32 via HWDGE sync (slow startup fine:
    # only the tail's fp32 pair-transposes read it).
    mp = {}
    ms = {}

    def load_mpair(n):
        p = mcpool.tile([TP, 2 * MW], bf16, tag=f"mp{n}", name=f"mp{n}")
        nc.gpsimd.dma_start(
            p.rearrange("p (c w) -> p c w", c=2),
            m_ap[9 + n:19 + n:9].rearrange("c (p t) f -> p c (t f)", p=TP),
        )
        mp[n] = p

    def load_msingle(n):
        s = mcpool.tile([TP, MW], bf16, tag=f"ms{n}", name=f"ms{n}")
        nc.gpsimd.dma_start(s[:], m_ap[n].rearrange("(p t) f -> p (t f)", p=TP))
        ms[n] = s

    load_mpair(0)
    identb = const.tile([128, 128], bf16, tag="identb")
    nc.gpsimd.dma_start(identb[:], id_ap)
    load_msingle(0)

    ident = const.tile([128, 128], f32, tag="ident")
    nc.sync.dma_start(ident[:], id_ap)

    xns = []
    for ci, (f0, fw) in enumerate(FCS):
        xn = xpool.tile([fw, (T + 2) * 2], bf16, tag=f"xn{f0}", name=f"xn{f0}")
        nc.vector.memset(xn[:, 0:4], 0.0)
        nc.gpsimd.dma_start(
            xn[:, 4:], x_ap[f0:f0 + fw].rearrange("f t c -> f (t c)")
        )
        xns.append(xn)

    for n in range(1, 9):
        load_mpair(n)
        load_msingle(n)

    # ---- x planes (bf16): f origin at col 1, zero pads at cols 0, 258, 259.
    # xr/xi are plain; xrs = +s*xr and xin = -s*xi carry the basis scale.
    planes = {}
    for nm in ("xr", "xi", "xrs", "xin"):
        p = const.tile([TP, PLW], bf16, tag=nm, name=nm)
        if nm in ("xr", "xi"):
            pv = p.rearrange("p (s w) -> p s w", w=SROW)
            nc.vector.memset(pv[:, :, 0:1], 0.0)
            nc.vector.memset(pv[:, :, 258:260], 0.0)
        planes[nm] = p

    # sideband (f=256) accumulators, one per complex component
    sbr = const.tile([TP, TAU], bf16, tag="sbr")
    sbi = const.tile([TP, TAU], bf16, tag="sbi")
    nc.vector.memset(sbr[:], 0.0)
    nc.vector.memset(sbi[:], 0.0)

    # ---- transpose x into the planes (fp32 PE transposes into psum banks).
    # Copies cast fp32->bf16: slot groups g0 on the idle DVE, g1/g2 on ACT.
    # Scaled planes built per group right after (DVE for g0, ACT for rest).
    bank_rr = 0
    for gi, (g0, gn) in enumerate(SLOT_GROUPS):
        for ci, (f0, fw) in enumerate(FCS):
            xn3 = xns[ci].rearrange("f (t c) -> f t c", c=2)
            for q, nm in enumerate(("xr", "xi")):
                pA = planes[nm].rearrange("p (s w) -> p s w", w=SROW)
                ptg = psum.tile(
                    [TP, 512], bf16, tag=BTAGS[bank_rr % 8], name="ptg",
                    padded_shape=[128, 1024],
                )
                bank_rr += 1
                for u in range(gn):
                    ts = g0 + u
                    nc.tensor.transpose(
                        ptg[0:TP, 128 * u:128 * u + fw],
                        xn3[0:fw, ts:ts + TAU * (TP - 1) + 1:TAU, q],
                        identb[0:fw, 0:fw],
                    )
                src = ptg.rearrange("p (u w) -> p u w", w=128)[0:TP, 0:gn, 0:fw]
                dst = pA[:, g0:g0 + gn, 1 + f0:1 + f0 + fw]
                if gi == 0:
                    nc.vector.tensor_copy(dst, src)
                else:
                    nc.scalar.copy(dst, src)
        # scaled planes for this slot group
        for src_nm, dst_nm, sc in (("xr", "xrs", SQ3H), ("xi", "xin", -SQ3H)):
            sv = planes[src_nm].rearrange("p (s w) -> p s w", w=SROW)
            dv = planes[dst_nm].rearrange("p (s w) -> p s w", w=SROW)
            if gi == 0:
                nc.vector.tensor_scalar_mul(
                    dv[:, g0:g0 + gn, :], sv[:, g0:g0 + gn, :], sc)
            else:
                nc.scalar.mul(dv[:, g0:g0 + gn, :], sv[:, g0:g0 + gn, :], sc)
    xpool.release()

    prep = ctx.enter_context(tc.tile_pool(name="prep", bufs=2))
    prod = ctx.enter_context(tc.tile_pool(name="prod", bufs=5))
    yop = ctx.enter_context(tc.tile_pool(name="yop", bufs=1))

    # ---- PSUM accumulators: accr rows of 256 in banks 0-3, acci in 4-7.
    # 512 fp32 = one bank = two tau rows.
    accr_c = [
        psum.tile([TP, 512], f32, tag=BTAGS[c], name=f"accr{c}",
                  padded_shape=[128, 512])
        for c in range(4)
    ]
    acci_c = [
        psum.tile([TP, 512], f32, tag=BTAGS[4 + c], name=f"acci{c}",
                  padded_shape=[128, 512])
        for c in range(4)
    ]

    idw = identb[0:TP, 0:TP]

    def prep_td(n, r0=0, rn=TAU, th_on_dve=False):
        """DVE half of prep: t1 = m9+m18, d = m9-m18 (tile-local rows).
        th = -0.5*t1 runs on ACT (one-tap lookahead) except when the ACT
        queue is backlogged (first tap, during the head copies)."""
        sl = slice(r0 * F, (r0 + rn) * F)
        m9 = mp[n][:, 0:MW][:, sl]
        m18 = mp[n][:, MW:2 * MW][:, sl]
        t1 = prep.tile([TP, rn * F], bf16, tag="t1", name="t1")
        nc.vector.tensor_add(t1[:], m9, m18)
        d = prep.tile([TP, rn * F], bf16, tag="d", name="d")
        nc.vector.tensor_sub(d[:], m9, m18)
        th = prep.tile([TP, rn * F], bf16, tag="th", name="th")
        if th_on_dve:
            nc.vector.tensor_scalar_mul(th[:], t1[:], -0.5)
        else:
            nc.scalar.mul(th[:], t1[:], -0.5)
        return th, d

    def prep_ar(n, th, r0=0, rn=TAU):
        ar = prep.tile([TP, rn * F], bf16, tag="ar", name="ar")
        nc.vector.tensor_add(ar[:], th[:], ms[n][:, r0 * F:(r0 + rn) * F])
        return ar

    def products(n, ar, d, r0, rn):
        """4 plain-mul product tiles for tau rows [r0, r0+rn) (tile-local)."""
        i, j = divmod(n, 3)

        def xv(nm):
            return planes[nm].rearrange("p (s w) -> p s w", w=SROW)[
                :, i + r0:i + r0 + rn, j:j + F]

        a8 = ar.rearrange("p (r w) -> p r w", w=F)
        d8 = d.rearrange("p (r w) -> p r w", w=F)
        w = rn * F
        p0 = prod.tile([TP, w], bf16, tag="P", name="p0")
        p1 = prod.tile([TP, w], bf16, tag="P", name="p1")
        p2 = prod.tile([TP, w], bf16, tag="P", name="p2")
        p3 = prod.tile([TP, w], bf16, tag="P", name="p3")
        nc.vector.tensor_mul(p0.rearrange("p (r w) -> p r w", w=F), a8, xv("xr"))
        nc.vector.tensor_mul(p1.rearrange("p (r w) -> p r w", w=F), a8, xv("xi"))
        nc.vector.tensor_mul(p2.rearrange("p (r w) -> p r w", w=F), d8, xv("xin"))
        nc.vector.tensor_mul(p3.rearrange("p (r w) -> p r w", w=F), d8, xv("xrs"))
        return p0, p1, p2, p3

    def accum_main(n, tiles, r0, rn, last):
        """PE-accumulate tau rows [r0, r0+rn) of the products into PSUM."""
        p0, p1, p2, p3 = tiles
        first = n == 0
        for c in range(r0 // 2, (r0 + rn) // 2):
            lo = c * 2 - r0
            for acc, pa, pb in ((accr_c[c], p0, p2), (acci_c[c], p1, p3)):
                for k, pt in enumerate((pa, pb)):
                    pv = pt.rearrange("p (r w) -> p r w", w=F)[
                        :, lo:lo + 2, 0:256]
                    nc.tensor.matmul(
                        acc[:], idw, pv,
                        start=(first and k == 0),
                        stop=(last and k == 1),
                    )

    def accum_sb(n, tiles, r0, rn):
        # tiny f=256 adds run on the mostly-idle GpSimd to keep the DVE
        # stream free of per-op semaphore overhead
        if n % 3 == 2:
            return
        p0, p1, p2, p3 = tiles
        for acc, pa, pb in ((sbr, p0, p2), (sbi, p1, p3)):
            for pt in (pa, pb):
                pv = pt.rearrange("p (r w) -> p r w", w=F)[:, 0:rn, 256]
                nc.gpsimd.tensor_add(acc[:, r0:r0 + rn], acc[:, r0:r0 + rn], pv)

    # drained accumulators in (f-major, tau-minor) bf16 layout: adjacent tau
    # pairs (t=8p+2q, +1) are then adjacent bytes, so the output transposes
    # can run on fp32-reinterpreted PAIRS (psum matmul writes need 4B align).
    acc_s = [
        const.tile([TP, AW], bf16, tag="accr_s", name="accr_s"),
        const.tile([TP, AW], bf16, tag="acci_s", name="acci_s"),
    ]
    acc32 = [a.bitcast(f32) for a in acc_s]

    yo01 = yop.tile([128, 2 * T * 2], bf16, tag="yo01", name="yo01")
    yviews = [
        yo01[:, 0:T * 2].rearrange("f (t c) -> f t c", c=2),
        yo01[:, T * 2:].rearrange("f (t c) -> f t c", c=2),
    ]

    # output collector psum banks: one per (f-half, comp); the 4 fp32 pair
    # transposes land strided (pair position = 4p+q fp32) into one bank,
    # leaving it t-contiguous bf16; then ONE copy into yo01.
    # Banks are reused in drain-retirement order (quarter c frees c and 4+c).
    OBANK = {(0, 0): 0, (0, 1): 4, (1, 0): 1, (1, 1): 5}
    obank = {}

    def drain_chunk(c):
        for comp, acc in ((0, accr_c[c]), (1, acci_c[c])):
            src = acc.rearrange("p (r f) -> p f r", r=2)
            dst = acc_s[comp].rearrange("p (f r) -> p f r", r=TAU)[
                :, :, 2 * c:2 * c + 2]
            nc.scalar.copy(dst, src)

    out_done = {k: 0 for k in OBANK}

    def out_rows(q):
        """After drain_chunk(q): transpose every tau PAIR that is both
        drained (pair <= q) and whose collector bank is retired (f0's
        banks 0/4 retire at q>=0, f1's banks 1/5 at q>=1)."""
        for ci in (0, 1):
            if q < ci:
                continue
            f0, fw = FCS[ci]
            for comp in (0, 1):
                key = (ci, comp)
                if key not in obank:
                    obank[key] = psum.tile(
                        [128, T // 2], f32, tag=BTAGS[OBANK[key]],
                        name=f"ob{ci}{comp}", padded_shape=[128, 512],
                    )
                ob = obank[key]
                a32 = acc32[comp].rearrange("p (f r) -> p f r", r=TAU // 2)
                for rp in range(out_done[key], q + 1):
                    # each strided transpose is its own single-matmul group
                    # (start=False accumulation into untouched elements is
                    # not safe; disjoint start=True writes are).
                    nc.tensor.matmul(
                        ob[0:fw, rp:rp + 4 * (TP - 1) + 1:4],
                        a32[:, f0:f0 + fw, rp],
                        ident[0:TP, 0:TP],
                        is_transpose=True,
                    )
                out_done[key] = q + 1

    for n in range(C // 3):
        if n == 0:
            th0, d0 = prep_td(0, th_on_dve=True)
            ar0, dd = prep_ar(0, th0), d0
        if n < 8:
            nxt_td = prep_td(n + 1)
            tiles = products(n, ar0, dd, 0, TAU)
            accum_main(n, tiles, 0, TAU, last=False)
            accum_sb(n, tiles, 0, TAU)
            ar0, dd = prep_ar(n + 1, nxt_td[0]), nxt_td[1]
        else:
            tiles = products(n, ar0, dd, 0, TAU)
            accum_main(n, tiles, 0, TAU, last=True)
            # drains + output transposes staged per retired psum chunk
            for q in range(4):
                drain_chunk(q)
                out_rows(q)

    # ---- one contiguous-psum-source copy per (f-half, comp), then stores.
    # DVE takes the real comps, ACT the imag; casting SWDGE stores per half.
    for ci in (0, 1):
        for comp in (0, 1):
            dst = yviews[ci][0:128, :, comp]
            src = obank[(ci, comp)].bitcast(bf16)[0:128, 0:T]
            if comp == 0:
                nc.vector.tensor_copy(dst, src)
            else:
                nc.scalar.copy(dst, src)
        nc.gpsimd.dma_start(
            y_ap[128 * ci:128 * (ci + 1)].rearrange("f t c -> f (t c)"),
            yo01[:, T * 2 * ci:T * 2 * (ci + 1)],
        )
    # f=256 sideband: interleave (t, c) as f32 in SBUF (2 tiny ACT copies),
    # then one contiguous-row store (64B runs per partition).
    sbri = yop.tile([TP, 2 * TAU], f32, tag="sbri", name="sbri")
    sbv = sbri.rearrange("p (t c) -> p t c", c=2)
    nc.scalar.copy(sbv[:, :, 0], sbr[:])
    nc.scalar.copy(sbv[:, :, 1], sbi[:])
    nc.gpsimd.dma_start(
        y_ap[256].rearrange("(p t) c -> p (t c)", p=TP), sbri[:]
    )


def _build():
    if "nc" in _CACHE:
        return _CACHE["nc"]
    from contextlib import ExitStack
    from concourse import bacc, mybir
    import concourse.tile as tile

    f32 = mybir.dt.float32
    nc = bacc.Bacc("TRN2", target_bir_lowering=False, debug=False, num_devices=B)
    m_d = nc.dram_tensor("m", (C, T, F), f32, kind="ExternalInput")
    x_d = nc.dram_tensor("x", (F, T, 2), f32, kind="ExternalInput")
    id_d = nc.dram_tensor("ident", (128, 128), f32, kind="ExternalInput")
    y_d = nc.dram_tensor("y", (F, T, 2), f32, kind="ExternalOutput")

    with tile.TileContext(nc) as tc:
        with ExitStack() as ctx:
            _emit(ctx, tc, m_d.ap(), x_d.ap(), id_d.ap(), y_d.ap())
    nc.compile()
    _CACHE["nc"] = nc
    return nc


def _in_maps(m, x):
    ident = np.eye(128, dtype=np.float32)
    return [
        {"m": np.ascontiguousarray(m[b]), "x": np.ascontiguousarray(x[b]),
         "ident": ident}
        for b in range(B)
    ]


def kernel(m, x, v, _trace=False):
    from concourse import bass_utils

    m = np.asarray(m, dtype=np.float32)
    x = np.asarray(x, dtype=np.float32)
    nc = _build()
    res = bass_utils.run_bass_kernel_spmd(
        nc, _in_maps(m, x), core_ids=list(range(B)), trace=_trace
    )
    kernel.last_results = res
    y = np.stack(
        [np.asarray(res.results[b]["y"], dtype=np.float32) for b in range(B)],
        axis=0,
    )
    return y


# revision 26
# speedup vs baseline: 1.1164x; 1.1164x over previous
"""Trainium2 Bass kernel for nn_CCM: per-pixel complex 3x3 conv mask.

Math (per batch element b, sharded 1 batch element per NeuronCore):
  y[t,f] = sum_{n=0..8} A_n[t,f] * X[t+i(n)-2, f+j(n)-1]   (complex)
with A_n = m_n + w * m_{9+n} + conj(w) * m_{18+n}, w = -1/2 + i*sqrt(3)/2:
  Ar_n = m_n - 0.5*(m_{9+n} + m_{18+n})
  Ai_n = s * (m_{9+n} - m_{18+n}),  s = sqrt(3)/2
X = xr + i*xi, zero padded (causal in t: 2 top; symmetric in f: 1,1).

v4 design (trace-driven, from v1-v3):
- The kernel is jointly limited by the 29.8MB fp32 load stream (~84us at
  358GB/s) and the DVE. Everything else is arranged to keep both saturated:
- Per-tap products accumulate into PSUM fp32 via identity-weight matmuls on
  the PE (removes all accumulation adds from the DVE; better precision than
  bf16 accumulators).
- The +-s basis scale lives in two extra SCALED x planes (xrs=+s*xr,
  xin=-s*xi); th=-0.5*(m9+m18) runs on ACT with one-tap lookahead so the
  DVE never waits. DVE per tap: t1, d, ar + 4 plain 2x tensor_tensor muls.
- x stages via HWDGE (sync) as fp32 so the SWDGE m-stream starts ~4us
  earlier; x transposes run in fp32, plane copies cast to bf16.
- PSUM holds 4096 fp32/partition but the output needs 2*2056 so the f=256
  column accumulates in SBUF via tiny strided DVE adds (j==2 taps are zero
  there and skipped); it stores via a direct scatter SWDGE cast DMA.
- Output: per (f-half, comp) ONE psum bank collects all 8 tau-row
  transposes (start=False accumulate-into-disjoint-elements), then ONE
  contiguous-source copy into the bf16 staging tile; casting SWDGE stores.
- Tap 8's m tiles load in tau-quarters so its products/dr ains/transposes
  pipeline with the last DMA arrivals.
- PSUM banks are time-shared via same-tag tile reuse: head x-transposes ->
  accumulators -> tail output collectors.
"""

import sys
import numpy as np

sys.path.insert(0, "/opt/trn_rl_repo")

B = 8
C = 27
T = 1000
F = 257
TP = 125          # partitions
TAU = 8           # t = 8*p + tau
NS = 10           # slots in x planes: tau in [-2, 8)
SROW = 260        # x plane slot row width (elements)
MW = TAU * F      # 2056: m / prod tile width (flat, rows of 257)
AW = TAU * 256    # 2048: psum accumulator width (rows of 256)
PLW = NS * SROW   # 2600: x plane width
QW = 2 * F        # 514: one tau-quarter of a flat m plane
SQ3H = float(np.sqrt(3.0) / 2.0)

_CACHE = {}


def _emit(ctx, tc, m_ap, x_ap, id_ap, y_ap):
    import concourse.mybir as mybir

    nc = tc.nc
    f32 = mybir.dt.float32
    bf16 = mybir.dt.bfloat16
    FCS = [(0, 128), (128, 128), (256, 1)]   # f chunks for transposes
    SLOT_GROUPS = [(0, 4), (4, 4), (8, 2)]   # batches of slots per psum tile

    const = ctx.enter_context(tc.tile_pool(name="const", bufs=1))
    mcpool = ctx.enter_context(tc.tile_pool(name="mc", bufs=1))
    xpool = tc.alloc_tile_pool(name="xstage", bufs=1)
    # One PSUM pool; the 8 banks rotate roles via tag reuse:
    # head ptg transposes -> accr/acci accumulators -> tail out collectors.
    psum = ctx.enter_context(tc.tile_pool(name="psum", bufs=1, space="PSUM"))
    BTAGS = [f"bank{i}" for i in range(8)]

    # ---- SWDGE casting-load order: tap 0's pair first (it gates the DVE's
    # first op), then identb (gates PE transposes), tap 0's single, then x,
    # then the remaining taps. ident f32 via HWDGE sync (slow startup fine:
    # only the tail's fp32 pair-transposes read it).
    mp = {}
    ms = {}

    def load_mpair(n):
        p = mcpool.tile([TP, 2 * MW], bf16, tag=f"mp{n}", name=f"mp{n}")
        nc.gpsimd.dma_start(
            p.rearrange("p (c w) -> p c w", c=2),
            m_ap[9 + n:19 + n:9].rearrange("c (p t) f -> p c (t f)", p=TP),
        )
        mp[n] = p

    def load_msingle(n):
        s = mcpool.tile([TP, MW], bf16, tag=f"ms{n}", name=f"ms{n}")
        nc.gpsimd.dma_start(s[:], m_ap[n].rearrange("(p t) f -> p (t f)", p=TP))
        ms[n] = s

    load_mpair(0)
    identb = const.tile([128, 128], bf16, tag="identb")
    nc.gpsimd.dma_start(identb[:], id_ap)
    load_msingle(0)

    ident = const.tile([128, 128], f32, tag="ident")
    nc.sync.dma_start(ident[:], id_ap)

    xns = []
    for ci, (f0, fw) in enumerate(FCS):
        xn = xpool.tile([fw, (T + 2) * 2], bf16, tag=f"xn{f0}", name=f"xn{f0}")
        nc.vector.memset(xn[:, 0:4], 0.0)
        nc.gpsimd.dma_start(
            xn[:, 4:], x_ap[f0:f0 + fw].rearrange("f t c -> f (t c)")
        )
        xns.append(xn)

    for n in range(1, 9):
        load_mpair(n)
        load_msingle(n)

    # ---- x planes (bf16): f origin at col 1, zero pads at cols 0, 258, 259.
    # xr/xi are plain; xrs = +s*xr and xin = -s*xi carry the basis scale.
    planes = {}
    for nm in ("xr", "xi", "xrs", "xin"):
        p = const.tile([TP, PLW], bf16, tag=nm, name=nm)
        if nm in ("xr", "xi"):
            pv = p.rearrange("p (s w) -> p s w", w=SROW)
            nc.vector.memset(pv[:, :, 0:1], 0.0)
            nc.vector.memset(pv[:, :, 258:260], 0.0)
        planes[nm] = p

    # sideband (f=256) accumulators, one per complex component
    sbr = const.tile([TP, TAU], bf16, tag="sbr")
    sbi = const.tile([TP, TAU], bf16, tag="sbi")
    nc.vector.memset(sbr[:], 0.0)
    nc.vector.memset(sbi[:], 0.0)

    # ---- transpose x into the planes (fp32 PE transposes into psum banks).
    # Copies cast fp32->bf16: slot groups g0 on the idle DVE, g1/g2 on ACT.
    # Scaled planes built per group right after (DVE for g0, ACT for rest).
    bank_rr = 0
    for gi, (g0, gn) in enumerate(SLOT_GROUPS):
        for ci, (f0, fw) in enumerate(FCS):
            xn3 = xns[ci].rearrange("f (t c) -> f t c", c=2)
            for q, nm in enumerate(("xr", "xi")):
                pA = planes[nm].rearrange("p (s w) -> p s w", w=SROW)
                ptg = psum.tile(
                    [TP, 512], bf16, tag=BTAGS[bank_rr % 8], name="ptg",
                    padded_shape=[128, 1024],
                )
                bank_rr += 1
                for u in range(gn):
                    ts = g0 + u
                    nc.tensor.transpose(
                        ptg[0:TP, 128 * u:128 * u + fw],
                        xn3[0:fw, ts:ts + TAU * (TP - 1) + 1:TAU, q],
                        identb[0:fw, 0:fw],
                    )
                src = ptg.rearrange("p (u w) -> p u w", w=128)[0:TP, 0:gn, 0:fw]
                dst = pA[:, g0:g0 + gn, 1 + f0:1 + f0 + fw]
                if gi == 0:
                    nc.vector.tensor_copy(dst, src)
                else:
                    nc.scalar.copy(dst, src)
        # scaled planes for this slot group
        for src_nm, dst_nm, sc in (("xr", "xrs", SQ3H), ("xi", "xin", -SQ3H)):
            sv = planes[src_nm].rearrange("p (s w) -> p s w", w=SROW)
            dv = planes[dst_nm].rearrange("p (s w) -> p s w", w=SROW)
            if gi == 0:
                nc.vector.tensor_scalar_mul(
                    dv[:, g0:g0 + gn, :], sv[:, g0:g0 + gn, :], sc)
            else:
                nc.scalar.mul(dv[:, g0:g0 + gn, :], sv[:, g0:g0 + gn, :], sc)
    xpool.release()

    prep = ctx.enter_context(tc.tile_pool(name="prep", bufs=2))
    prod = ctx.enter_context(tc.tile_pool(name="prod", bufs=5))
    yop = ctx.enter_context(tc.tile_pool(name="yop", bufs=1))

    # ---- PSUM accumulators: accr rows of 256 in banks 0-3, acci in 4-7.
    # 512 fp32 = one bank = two tau rows.
    accr_c = [
        psum.tile([TP, 512], f32, tag=BTAGS[c], name=f"accr{c}",
                  padded_shape=[128, 512])
        for c in range(4)
    ]
    acci_c = [
        psum.tile([TP, 512], f32, tag=BTAGS[4 + c], name=f"acci{c}",
                  padded_shape=[128, 512])
        for c in range(4)
    ]

    idw = identb[0:TP, 0:TP]

    def prep_td(n, r0=0, rn=TAU, th_on_dve=False):
        """DVE half of prep: t1 = m9+m18, d = m9-m18 (tile-local rows).
        th = -0.5*t1 runs on ACT (one-tap lookahead) except when the ACT
        queue is backlogged (first tap, during the head copies)."""
        sl = slice(r0 * F, (r0 + rn) * F)
        m9 = mp[n][:, 0:MW][:, sl]
        m18 = mp[n][:, MW:2 * MW][:, sl]
        t1 = prep.tile([TP, rn * F], bf16, tag="t1", name="t1")
        nc.vector.tensor_add(t1[:], m9, m18)
        d = prep.tile([TP, rn * F], bf16, tag="d", name="d")
        nc.vector.tensor_sub(d[:], m9, m18)
        th = prep.tile([TP, rn * F], bf16, tag="th", name="th")
        if th_on_dve:
            nc.vector.tensor_scalar_mul(th[:], t1[:], -0.5)
        else:
            nc.scalar.mul(th[:], t1[:], -0.5)
        return th, d

    def prep_ar(n, th, r0=0, rn=TAU):
        ar = prep.tile([TP, rn * F], bf16, tag="ar", name="ar")
        nc.vector.tensor_add(ar[:], th[:], ms[n][:, r0 * F:(r0 + rn) * F])
        return ar

    def products(n, ar, d, r0, rn):
        """4 plain-mul product tiles for tau rows [r0, r0+rn) (tile-local)."""
        i, j = divmod(n, 3)

        def xv(nm):
            return planes[nm].rearrange("p (s w) -> p s w", w=SROW)[
                :, i + r0:i + r0 + rn, j:j + F]

        a8 = ar.rearrange("p (r w) -> p r w", w=F)
        d8 = d.rearrange("p (r w) -> p r w", w=F)
        w = rn * F
        p0 = prod.tile([TP, w], bf16, tag="P", name="p0")
        p1 = prod.tile([TP, w], bf16, tag="P", name="p1")
        p2 = prod.tile([TP, w], bf16, tag="P", name="p2")
        p3 = prod.tile([TP, w], bf16, tag="P", name="p3")
        nc.vector.tensor_mul(p0.rearrange("p (r w) -> p r w", w=F), a8, xv("xr"))
        nc.vector.tensor_mul(p1.rearrange("p (r w) -> p r w", w=F), a8, xv("xi"))
        nc.vector.tensor_mul(p2.rearrange("p (r w) -> p r w", w=F), d8, xv("xin"))
        nc.vector.tensor_mul(p3.rearrange("p (r w) -> p r w", w=F), d8, xv("xrs"))
        return p0, p1, p2, p3

    def accum_main(n, tiles, r0, rn, last):
        """PE-accumulate tau rows [r0, r0+rn) of the products into PSUM."""
        p0, p1, p2, p3 = tiles
        first = n == 0
        for c in range(r0 // 2, (r0 + rn) // 2):
            lo = c * 2 - r0
            for acc, pa, pb in ((accr_c[c], p0, p2), (acci_c[c], p1, p3)):
                for k, pt in enumerate((pa, pb)):
                    pv = pt.rearrange("p (r w) -> p r w", w=F)[
                        :, lo:lo + 2, 0:256]
                    nc.tensor.matmul(
                        acc[:], idw, pv,
                        start=(first and k == 0),
                        stop=(last and k == 1),
                    )

    def accum_sb(n, tiles, r0, rn):
        if n % 3 == 2:
            return
        p0, p1, p2, p3 = tiles
        for acc, pa, pb in ((sbr, p0, p2), (sbi, p1, p3)):
            for pt in (pa, pb):
                pv = pt.rearrange("p (r w) -> p r w", w=F)[:, 0:rn, 256]
                nc.vector.tensor_add(acc[:, r0:r0 + rn], acc[:, r0:r0 + rn], pv)

    # drained accumulators in (f-major, tau-minor) bf16 layout: adjacent tau
    # pairs (t=8p+2q, +1) are then adjacent bytes, so the output transposes
    # can run on fp32-reinterpreted PAIRS (psum matmul writes need 4B align).
    acc_s = [
        const.tile([TP, AW], bf16, tag="accr_s", name="accr_s"),
        const.tile([TP, AW], bf16, tag="acci_s", name="acci_s"),
    ]
    acc32 = [a.bitcast(f32) for a in acc_s]

    yo01 = yop.tile([128, 2 * T * 2], bf16, tag="yo01", name="yo01")
    yviews = [
        yo01[:, 0:T * 2].rearrange("f (t c) -> f t c", c=2),
        yo01[:, T * 2:].rearrange("f (t c) -> f t c", c=2),
    ]

    # output collector psum banks: one per (f-half, comp); the 4 fp32 pair
    # transposes land strided (pair position = 4p+q fp32) into one bank,
    # leaving it t-contiguous bf16; then ONE copy into yo01.
    # Banks are reused in drain-retirement order (quarter c frees c and 4+c).
    OBANK = {(0, 0): 0, (0, 1): 4, (1, 0): 1, (1, 1): 5}
    obank = {}

    def drain_chunk(c):
        for comp, acc in ((0, accr_c[c]), (1, acci_c[c])):
            src = acc.rearrange("p (r f) -> p f r", r=2)
            dst = acc_s[comp].rearrange("p (f r) -> p f r", r=TAU)[
                :, :, 2 * c:2 * c + 2]
            nc.scalar.copy(dst, src)

    out_done = {k: 0 for k in OBANK}

    def out_rows(q):
        """After drain_chunk(q): transpose every tau PAIR that is both
        drained (pair <= q) and whose collector bank is retired (f0's
        banks 0/4 retire at q>=0, f1's banks 1/5 at q>=1)."""
        for ci in (0, 1):
            if q < ci:
                continue
            f0, fw = FCS[ci]
            for comp in (0, 1):
                key = (ci, comp)
                if key not in obank:
                    obank[key] = psum.tile(
                        [128, T // 2], f32, tag=BTAGS[OBANK[key]],
                        name=f"ob{ci}{comp}", padded_shape=[128, 512],
                    )
                ob = obank[key]
                a32 = acc32[comp].rearrange("p (f r) -> p f r", r=TAU // 2)
                for rp in range(out_done[key], q + 1):
                    # each strided transpose is its own single-matmul group
                    # (start=False accumulation into untouched elements is
                    # not safe; disjoint start=True writes are).
                    nc.tensor.matmul(
                        ob[0:fw, rp:rp + 4 * (TP - 1) + 1:4],
                        a32[:, f0:f0 + fw, rp],
                        ident[0:TP, 0:TP],
                        is_transpose=True,
                    )
                out_done[key] = q + 1

    for n in range(C // 3):
        if n == 0:
            th0, d0 = prep_td(0, th_on_dve=True)
            ar0, dd = prep_ar(0, th0), d0
        if n < 8:
            nxt_td = prep_td(n + 1)
            tiles = products(n, ar0, dd, 0, TAU)
            accum_main(n, tiles, 0, TAU, last=False)
            accum_sb(n, tiles, 0, TAU)
            ar0, dd = prep_ar(n + 1, nxt_td[0]), nxt_td[1]
        else:
            tiles = products(n, ar0, dd, 0, TAU)
            accum_main(n, tiles, 0, TAU, last=True)
            # drains + output transposes staged per retired psum chunk
            for q in range(4):
                drain_chunk(q)
                out_rows(q)

    # ---- one contiguous-psum-source copy per (f-half, comp), then stores.
    # DVE takes the real comps, ACT the imag; casting SWDGE stores per half.
    for ci in (0, 1):
        for comp in (0, 1):
            dst = yviews[ci][0:128, :, comp]
            src = obank[(ci, comp)].bitcast(bf16)[0:128, 0:T]
            if comp == 0:
                nc.vector.tensor_copy(dst, src)
            else:
                nc.scalar.copy(dst, src)
        nc.gpsimd.dma_start(
            y_ap[128 * ci:128 * (ci + 1)].rearrange("f t c -> f (t c)"),
            yo01[:, T * 2 * ci:T * 2 * (ci + 1)],
        )
    # f=256 sideband: interleave (t, c) as f32 in SBUF (2 tiny ACT copies),
    # then one contiguous-row store (64B runs per partition).
    sbri = yop.tile([TP, 2 * TAU], f32, tag="sbri", name="sbri")
    sbv = sbri.rearrange("p (t c) -> p t c", c=2)
    nc.scalar.copy(sbv[:, :, 0], sbr[:])
    nc.scalar.copy(sbv[:, :, 1], sbi[:])
    nc.gpsimd.dma_start(
        y_ap[256].rearrange("(p t) c -> p (t c)", p=TP), sbri[:]
    )


def _build():
    if "nc" in _CACHE:
        return _CACHE["nc"]
    from contextlib import ExitStack
    from concourse import bacc, mybir
    import concourse.tile as tile

    f32 = mybir.dt.float32
    nc = bacc.Bacc("TRN2", target_bir_lowering=False, debug=False, num_devices=B)
    m_d = nc.dram_tensor("m", (C, T, F), f32, kind="ExternalInput")
    x_d = nc.dram_tensor("x", (F, T, 2), f32, kind="ExternalInput")
    id_d = nc.dram_tensor("ident", (128, 128), f32, kind="ExternalInput")
    y_d = nc.dram_tensor("y", (F, T, 2), f32, kind="ExternalOutput")

    with tile.TileContext(nc) as tc:
        with ExitStack() as ctx:
            _emit(ctx, tc, m_d.ap(), x_d.ap(), id_d.ap(), y_d.ap())
    nc.compile()
    _CACHE["nc"] = nc
    return nc


def _in_maps(m, x):
    ident = np.eye(128, dtype=np.float32)
    return [
        {"m": np.ascontiguousarray(m[b]), "x": np.ascontiguousarray(x[b]),
         "ident": ident}
        for b in range(B)
    ]


def kernel(m, x, v, _trace=False):
    from concourse import bass_utils

    m = np.asarray(m, dtype=np.float32)
    x = np.asarray(x, dtype=np.float32)
    nc = _build()
    res = bass_utils.run_bass_kernel_spmd(
        nc, _in_maps(m, x), core_ids=list(range(B)), trace=_trace
    )
    kernel.last_results = res
    y = np.stack(
        [np.asarray(res.results[b]["y"], dtype=np.float32) for b in range(B)],
        axis=0,
    )
    return y


# revision 30
# speedup vs baseline: 1.1570x; 1.0364x over previous
"""Trainium2 Bass kernel for nn_CCM: per-pixel complex 3x3 conv mask.

Math (per batch element b, sharded 1 batch element per NeuronCore):
  y[t,f] = sum_{n=0..8} A_n[t,f] * X[t+i(n)-2, f+j(n)-1]   (complex)
with A_n = m_n + w * m_{9+n} + conj(w) * m_{18+n}, w = -1/2 + i*sqrt(3)/2:
  Ar_n = m_n - 0.5*(m_{9+n} + m_{18+n})
  Ai_n = s * (m_{9+n} - m_{18+n}),  s = sqrt(3)/2
X = xr + i*xi, zero padded (causal in t: 2 top; symmetric in f: 1,1).

v4 design (trace-driven, from v1-v3):
- The kernel is jointly limited by the 29.8MB fp32 load stream (~84us at
  358GB/s) and the DVE. Everything else is arranged to keep both saturated:
- Per-tap products accumulate into PSUM fp32 via identity-weight matmuls on
  the PE (removes all accumulation adds from the DVE; better precision than
  bf16 accumulators).
- The +-s basis scale lives in two extra SCALED x planes (xrs=+s*xr,
  xin=-s*xi); th=-0.5*(m9+m18) runs on ACT with one-tap lookahead so the
  DVE never waits. DVE per tap: t1, d, ar + 4 plain 2x tensor_tensor muls.
- x stages via HWDGE (sync) as fp32 so the SWDGE m-stream starts ~4us
  earlier; x transposes run in fp32, plane copies cast to bf16.
- PSUM holds 4096 fp32/partition but the output needs 2*2056 so the f=256
  column accumulates in SBUF via tiny strided DVE adds (j==2 taps are zero
  there and skipped); it stores via a direct scatter SWDGE cast DMA.
- Output: per (f-half, comp) ONE psum bank collects all 8 tau-row
  transposes (start=False accumulate-into-disjoint-elements), then ONE
  contiguous-source copy into the bf16 staging tile; casting SWDGE stores.
- Tap 8's m tiles load in tau-quarters so its products/dr ains/transposes
  pipeline with the last DMA arrivals.
- PSUM banks are time-shared via same-tag tile reuse: head x-transposes ->
  accumulators -> tail output collectors.
"""

import sys
import numpy as np

sys.path.insert(0, "/opt/trn_rl_repo")

B = 8
C = 27
T = 1000
F = 257
TP = 125          # partitions
TAU = 8           # t = 8*p + tau
NS = 10           # slots in x planes: tau in [-2, 8)
SROW = 260        # x plane slot row width (elements)
MW = TAU * F      # 2056: m / prod tile width (flat, rows of 257)
AW = TAU * 256    # 2048: psum accumulator width (rows of 256)
PLW = NS * SROW   # 2600: x plane width
QW = 2 * F        # 514: one tau-quarter of a flat m plane
SQ3H = float(np.sqrt(3.0) / 2.0)

_CACHE = {}


def _emit(ctx, tc, m_ap, x_ap, id_ap, y_ap):
    import concourse.mybir as mybir

    nc = tc.nc
    f32 = mybir.dt.float32
    bf16 = mybir.dt.bfloat16
    FCS = [(0, 128), (128, 128), (256, 1)]   # f chunks for transposes
    SLOT_GROUPS = [(0, 4), (4, 4), (8, 2)]   # batches of slots per psum tile

    const = ctx.enter_context(tc.tile_pool(name="const", bufs=1))
    mcpool = ctx.enter_context(tc.tile_pool(name="mc", bufs=1))
    xpool = tc.alloc_tile_pool(name="xstage", bufs=1)
    # One PSUM pool; the 8 banks rotate roles via tag reuse:
    # head ptg transposes -> accr/acci accumulators -> tail out collectors.
    psum = ctx.enter_context(tc.tile_pool(name="psum", bufs=1, space="PSUM"))
    BTAGS = [f"bank{i}" for i in range(8)]

    # ---- SWDGE casting-load order: tap 0's pair first (it gates the DVE's
    # first op), then identb (gates PE transposes), tap 0's single, then x,
    # then the remaining taps. ident f32 via HWDGE sync (slow startup fine:
    # only the tail's fp32 pair-transposes read it).
    mp = {}
    ms = {}

    def load_mpair(n):
        p = mcpool.tile([TP, 2 * MW], bf16, tag=f"mp{n}", name=f"mp{n}")
        nc.gpsimd.dma_start(
            p.rearrange("p (c w) -> p c w", c=2),
            m_ap[9 + n:19 + n:9].rearrange("c (p t) f -> p c (t f)", p=TP),
        )
        mp[n] = p

    def load_msingle(n):
        s = mcpool.tile([TP, MW], bf16, tag=f"ms{n}", name=f"ms{n}")
        nc.gpsimd.dma_start(s[:], m_ap[n].rearrange("(p t) f -> p (t f)", p=TP))
        ms[n] = s

    identb = const.tile([128, 128], bf16, tag="identb")
    nc.gpsimd.dma_start(identb[:], id_ap)

    ident = const.tile([128, 128], f32, tag="ident")
    nc.sync.dma_start(ident[:], id_ap)

    xns = []
    for ci, (f0, fw) in enumerate(FCS):
        xn = xpool.tile([fw, (T + 2) * 2], bf16, tag=f"xn{f0}", name=f"xn{f0}")
        nc.vector.memset(xn[:, 0:4], 0.0)
        nc.gpsimd.dma_start(
            xn[:, 4:], x_ap[f0:f0 + fw].rearrange("f t c -> f (t c)")
        )
        xns.append(xn)

    for n in range(9):
        load_mpair(n)
        load_msingle(n)

    # ---- x planes (bf16): f origin at col 1, zero pads at cols 0, 258, 259.
    # xr/xi are plain; xrs = +s*xr and xin = -s*xi carry the basis scale.
    planes = {}
    for nm in ("xr", "xi", "xrs", "xin"):
        p = const.tile([TP, PLW], bf16, tag=nm, name=nm)
        if nm in ("xr", "xi"):
            pv = p.rearrange("p (s w) -> p s w", w=SROW)
            nc.vector.memset(pv[:, :, 0:1], 0.0)
            nc.vector.memset(pv[:, :, 258:260], 0.0)
        planes[nm] = p

    # sideband (f=256) accumulators, one per complex component
    sbr = const.tile([TP, TAU], bf16, tag="sbr")
    sbi = const.tile([TP, TAU], bf16, tag="sbi")
    nc.vector.memset(sbr[:], 0.0)
    nc.vector.memset(sbi[:], 0.0)

    # ---- transpose x into the planes (fp32 PE transposes into psum banks).
    # Copies cast fp32->bf16: slot groups g0 on the idle DVE, g1/g2 on ACT.
    # Scaled planes built per group right after (DVE for g0, ACT for rest).
    bank_rr = 0
    for gi, (g0, gn) in enumerate(SLOT_GROUPS):
        for ci, (f0, fw) in enumerate(FCS):
            xn3 = xns[ci].rearrange("f (t c) -> f t c", c=2)
            for q, nm in enumerate(("xr", "xi")):
                pA = planes[nm].rearrange("p (s w) -> p s w", w=SROW)
                ptg = psum.tile(
                    [TP, 512], bf16, tag=BTAGS[bank_rr % 8], name="ptg",
                    padded_shape=[128, 1024],
                )
                bank_rr += 1
                for u in range(gn):
                    ts = g0 + u
                    nc.tensor.transpose(
                        ptg[0:TP, 128 * u:128 * u + fw],
                        xn3[0:fw, ts:ts + TAU * (TP - 1) + 1:TAU, q],
                        identb[0:fw, 0:fw],
                    )
                src = ptg.rearrange("p (u w) -> p u w", w=128)[0:TP, 0:gn, 0:fw]
                dst = pA[:, g0:g0 + gn, 1 + f0:1 + f0 + fw]
                if gi == 0:
                    nc.vector.tensor_copy(dst, src)
                else:
                    nc.scalar.copy(dst, src)
        # scaled planes for this slot group
        for src_nm, dst_nm, sc in (("xr", "xrs", SQ3H), ("xi", "xin", -SQ3H)):
            sv = planes[src_nm].rearrange("p (s w) -> p s w", w=SROW)
            dv = planes[dst_nm].rearrange("p (s w) -> p s w", w=SROW)
            if gi == 0:
                nc.vector.tensor_scalar_mul(
                    dv[:, g0:g0 + gn, :], sv[:, g0:g0 + gn, :], sc)
            else:
                nc.scalar.mul(dv[:, g0:g0 + gn, :], sv[:, g0:g0 + gn, :], sc)
    xpool.release()

    prep = ctx.enter_context(tc.tile_pool(name="prep", bufs=2))
    prod = ctx.enter_context(tc.tile_pool(name="prod", bufs=5))
    yop = ctx.enter_context(tc.tile_pool(name="yop", bufs=1))

    # ---- PSUM accumulators: accr rows of 256 in banks 0-3, acci in 4-7.
    # 512 fp32 = one bank = two tau rows.
    accr_c = [
        psum.tile([TP, 512], f32, tag=BTAGS[c], name=f"accr{c}",
                  padded_shape=[128, 512])
        for c in range(4)
    ]
    acci_c = [
        psum.tile([TP, 512], f32, tag=BTAGS[4 + c], name=f"acci{c}",
                  padded_shape=[128, 512])
        for c in range(4)
    ]

    idw = identb[0:TP, 0:TP]

    def prep_td(n, r0=0, rn=TAU, th_on_dve=False):
        """DVE half of prep: t1 = m9+m18, d = m9-m18 (tile-local rows).
        th = -0.5*t1 runs on ACT (one-tap lookahead) except when the ACT
        queue is backlogged (first tap, during the head copies)."""
        sl = slice(r0 * F, (r0 + rn) * F)
        m9 = mp[n][:, 0:MW][:, sl]
        m18 = mp[n][:, MW:2 * MW][:, sl]
        t1 = prep.tile([TP, rn * F], bf16, tag="t1", name="t1")
        nc.vector.tensor_add(t1[:], m9, m18)
        d = prep.tile([TP, rn * F], bf16, tag="d", name="d", bufs=1)
        nc.vector.tensor_sub(d[:], m9, m18)
        th = prep.tile([TP, rn * F], bf16, tag="th", name="th")
        if th_on_dve:
            nc.vector.tensor_scalar_mul(th[:], t1[:], -0.5)
        else:
            nc.scalar.mul(th[:], t1[:], -0.5)
        return th, d

    def prep_ar(n, th, r0=0, rn=TAU):
        ar = prep.tile([TP, rn * F], bf16, tag="ar", name="ar", bufs=1)
        nc.vector.tensor_add(ar[:], th[:], ms[n][:, r0 * F:(r0 + rn) * F])
        return ar

    def products(n, ar, d, r0, rn):
        """4 plain-mul product tiles for tau rows [r0, r0+rn) (tile-local)."""
        i, j = divmod(n, 3)

        def xv(nm):
            return planes[nm].rearrange("p (s w) -> p s w", w=SROW)[
                :, i + r0:i + r0 + rn, j:j + F]

        a8 = ar.rearrange("p (r w) -> p r w", w=F)
        d8 = d.rearrange("p (r w) -> p r w", w=F)
        w = rn * F
        p0 = prod.tile([TP, w], bf16, tag="P", name="p0")
        p1 = prod.tile([TP, w], bf16, tag="P", name="p1")
        p2 = prod.tile([TP, w], bf16, tag="P", name="p2")
        p3 = prod.tile([TP, w], bf16, tag="P", name="p3")
        nc.vector.tensor_mul(p0.rearrange("p (r w) -> p r w", w=F), a8, xv("xr"))
        nc.vector.tensor_mul(p1.rearrange("p (r w) -> p r w", w=F), a8, xv("xi"))
        nc.vector.tensor_mul(p2.rearrange("p (r w) -> p r w", w=F), d8, xv("xin"))
        nc.vector.tensor_mul(p3.rearrange("p (r w) -> p r w", w=F), d8, xv("xrs"))
        return p0, p1, p2, p3

    def accum_main(n, tiles, r0, rn, last):
        """PE-accumulate tau rows [r0, r0+rn) of the products into PSUM."""
        p0, p1, p2, p3 = tiles
        first = n == 0
        for c in range(r0 // 2, (r0 + rn) // 2):
            lo = c * 2 - r0
            for acc, pa, pb in ((accr_c[c], p0, p2), (acci_c[c], p1, p3)):
                for k, pt in enumerate((pa, pb)):
                    pv = pt.rearrange("p (r w) -> p r w", w=F)[
                        :, lo:lo + 2, 0:256]
                    nc.tensor.matmul(
                        acc[:], idw, pv,
                        start=(first and k == 0),
                        stop=(last and k == 1),
                    )

    def accum_sb(n, tiles, r0, rn):
        if n % 3 == 2:
            return
        p0, p1, p2, p3 = tiles
        for acc, pa, pb in ((sbr, p0, p2), (sbi, p1, p3)):
            for pt in (pa, pb):
                pv = pt.rearrange("p (r w) -> p r w", w=F)[:, 0:rn, 256]
                nc.vector.tensor_add(acc[:, r0:r0 + rn], acc[:, r0:r0 + rn], pv)

    # drained accumulators in (f-major, tau-minor) bf16 layout: adjacent tau
    # pairs (t=8p+2q, +1) are then adjacent bytes, so the output transposes
    # can run on fp32-reinterpreted PAIRS (psum matmul writes need 4B align).
    acc_s = [
        const.tile([TP, AW], bf16, tag="accr_s", name="accr_s"),
        const.tile([TP, AW], bf16, tag="acci_s", name="acci_s"),
    ]
    acc32 = [a.bitcast(f32) for a in acc_s]

    yo01 = yop.tile([128, 2 * T * 2], f32, tag="yo01", name="yo01")
    yviews = [
        yo01[:, 0:T * 2].rearrange("f (t c) -> f t c", c=2),
        yo01[:, T * 2:].rearrange("f (t c) -> f t c", c=2),
    ]

    # output collector psum banks: one per (f-half, comp); the 4 fp32 pair
    # transposes land strided (pair position = 4p+q fp32) into one bank,
    # leaving it t-contiguous bf16; then ONE copy into yo01.
    # Banks are reused in drain-retirement order (quarter c frees c and 4+c).
    OBANK = {(0, 0): 0, (0, 1): 4, (1, 0): 1, (1, 1): 5}
    obank = {}

    def drain_chunk(c):
        for comp, acc in ((0, accr_c[c]), (1, acci_c[c])):
            src = acc.rearrange("p (r f) -> p f r", r=2)
            dst = acc_s[comp].rearrange("p (f r) -> p f r", r=TAU)[
                :, :, 2 * c:2 * c + 2]
            nc.scalar.copy(dst, src)

    out_done = {k: 0 for k in OBANK}

    def out_rows(q):
        """After drain_chunk(q): transpose every tau PAIR that is both
        drained (pair <= q) and whose collector bank is retired (f0's
        banks 0/4 retire at q>=0, f1's banks 1/5 at q>=1)."""
        for ci in (0, 1):
            if q < ci:
                continue
            f0, fw = FCS[ci]
            for comp in (0, 1):
                key = (ci, comp)
                if key not in obank:
                    obank[key] = psum.tile(
                        [128, T // 2], f32, tag=BTAGS[OBANK[key]],
                        name=f"ob{ci}{comp}", padded_shape=[128, 512],
                    )
                ob = obank[key]
                a32 = acc32[comp].rearrange("p (f r) -> p f r", r=TAU // 2)
                for rp in range(out_done[key], q + 1):
                    # each strided transpose is its own single-matmul group
                    # (start=False accumulation into untouched elements is
                    # not safe; disjoint start=True writes are).
                    nc.tensor.matmul(
                        ob[0:fw, rp:rp + 4 * (TP - 1) + 1:4],
                        a32[:, f0:f0 + fw, rp],
                        ident[0:TP, 0:TP],
                        is_transpose=True,
                    )
                out_done[key] = q + 1

    for n in range(C // 3):
        if n == 0:
            th0, d0 = prep_td(0, th_on_dve=True)
            ar0, dd = prep_ar(0, th0), d0
        if n < 8:
            nxt_td = prep_td(n + 1)
            tiles = products(n, ar0, dd, 0, TAU)
            accum_main(n, tiles, 0, TAU, last=False)
            accum_sb(n, tiles, 0, TAU)
            ar0, dd = prep_ar(n + 1, nxt_td[0]), nxt_td[1]
        else:
            tiles = products(n, ar0, dd, 0, TAU)
            accum_main(n, tiles, 0, TAU, last=True)
            # drains + output transposes staged per retired psum chunk
            for q in range(4):
                drain_chunk(q)
                out_rows(q)

    # ---- one contiguous-psum-source copy per (f-half, comp), then stores.
    # DVE takes the real comps, ACT the imag; casting SWDGE stores per half.
    for ci in (0, 1):
        for comp in (0, 1):
            dst = yviews[ci][0:128, :, comp]
            src = obank[(ci, comp)].bitcast(bf16)[0:128, 0:T]
            if comp == 0:
                nc.vector.tensor_copy(dst, src)
            else:
                nc.scalar.copy(dst, src)
        nc.gpsimd.dma_start(
            y_ap[128 * ci:128 * (ci + 1)].rearrange("f t c -> f (t c)"),
            yo01[:, T * 2 * ci:T * 2 * (ci + 1)],
        )
    # f=256 sideband: interleave (t, c) as f32 in SBUF (2 tiny ACT copies),
    # then one contiguous-row store (64B runs per partition).
    sbri = yop.tile([TP, 2 * TAU], f32, tag="sbri", name="sbri")
    sbv = sbri.rearrange("p (t c) -> p t c", c=2)
    nc.scalar.copy(sbv[:, :, 0], sbr[:])
    nc.scalar.copy(sbv[:, :, 1], sbi[:])
    nc.gpsimd.dma_start(
        y_ap[256].rearrange("(p t) c -> p (t c)", p=TP), sbri[:]
    )


def _build():
    if "nc" in _CACHE:
        return _CACHE["nc"]
    from contextlib import ExitStack
    from concourse import bacc, mybir
    import concourse.tile as tile

    f32 = mybir.dt.float32
    nc = bacc.Bacc("TRN2", target_bir_lowering=False, debug=False, num_devices=B)
    m_d = nc.dram_tensor("m", (C, T, F), f32, kind="ExternalInput")
    x_d = nc.dram_tensor("x", (F, T, 2), f32, kind="ExternalInput")
    id_d = nc.dram_tensor("ident", (128, 128), f32, kind="ExternalInput")
    y_d = nc.dram_tensor("y", (F, T, 2), f32, kind="ExternalOutput")

    with tile.TileContext(nc) as tc:
        with ExitStack() as ctx:
            _emit(ctx, tc, m_d.ap(), x_d.ap(), id_d.ap(), y_d.ap())
    nc.compile()
    _CACHE["nc"] = nc
    return nc


def _in_maps(m, x):
    ident = np.eye(128, dtype=np.float32)
    return [
        {"m": np.ascontiguousarray(m[b]), "x": np.ascontiguousarray(x[b]),
         "ident": ident}
        for b in range(B)
    ]


def kernel(m, x, v, _trace=False):
    from concourse import bass_utils

    m = np.asarray(m, dtype=np.float32)
    x = np.asarray(x, dtype=np.float32)
    nc = _build()
    res = bass_utils.run_bass_kernel_spmd(
        nc, _in_maps(m, x), core_ids=list(range(B)), trace=_trace
    )
    kernel.last_results = res
    y = np.stack(
        [np.asarray(res.results[b]["y"], dtype=np.float32) for b in range(B)],
        axis=0,
    )
    return y


# revision 31
# speedup vs baseline: 1.1639x; 1.0059x over previous
"""Trainium2 Bass kernel for nn_CCM: per-pixel complex 3x3 conv mask.

Math (per batch element b, sharded 1 batch element per NeuronCore):
  y[t,f] = sum_{n=0..8} A_n[t,f] * X[t+i(n)-2, f+j(n)-1]   (complex)
with A_n = m_n + w * m_{9+n} + conj(w) * m_{18+n}, w = -1/2 + i*sqrt(3)/2:
  Ar_n = m_n - 0.5*(m_{9+n} + m_{18+n})
  Ai_n = s * (m_{9+n} - m_{18+n}),  s = sqrt(3)/2
X = xr + i*xi, zero padded (causal in t: 2 top; symmetric in f: 1,1).

v4 design (trace-driven, from v1-v3):
- The kernel is jointly limited by the 29.8MB fp32 load stream (~84us at
  358GB/s) and the DVE. Everything else is arranged to keep both saturated:
- Per-tap products accumulate into PSUM fp32 via identity-weight matmuls on
  the PE (removes all accumulation adds from the DVE; better precision than
  bf16 accumulators).
- The +-s basis scale lives in two extra SCALED x planes (xrs=+s*xr,
  xin=-s*xi); th=-0.5*(m9+m18) runs on ACT with one-tap lookahead so the
  DVE never waits. DVE per tap: t1, d, ar + 4 plain 2x tensor_tensor muls.
- x stages via HWDGE (sync) as fp32 so the SWDGE m-stream starts ~4us
  earlier; x transposes run in fp32, plane copies cast to bf16.
- PSUM holds 4096 fp32/partition but the output needs 2*2056 so the f=256
  column accumulates in SBUF via tiny strided DVE adds (j==2 taps are zero
  there and skipped); it stores via a direct scatter SWDGE cast DMA.
- Output: per (f-half, comp) ONE psum bank collects all 8 tau-row
  transposes (start=False accumulate-into-disjoint-elements), then ONE
  contiguous-source copy into the bf16 staging tile; casting SWDGE stores.
- Tap 8's m tiles load in tau-quarters so its products/dr ains/transposes
  pipeline with the last DMA arrivals.
- PSUM banks are time-shared via same-tag tile reuse: head x-transposes ->
  accumulators -> tail output collectors.
"""

import sys
import numpy as np

sys.path.insert(0, "/opt/trn_rl_repo")

B = 8
C = 27
T = 1000
F = 257
TP = 125          # partitions
TAU = 8           # t = 8*p + tau
NS = 10           # slots in x planes: tau in [-2, 8)
SROW = 260        # x plane slot row width (elements)
MW = TAU * F      # 2056: m / prod tile width (flat, rows of 257)
AW = TAU * 256    # 2048: psum accumulator width (rows of 256)
PLW = NS * SROW   # 2600: x plane width
QW = 2 * F        # 514: one tau-quarter of a flat m plane
SQ3H = float(np.sqrt(3.0) / 2.0)

_CACHE = {}


def _emit(ctx, tc, m_ap, x_ap, id_ap, y_ap):
    import concourse.mybir as mybir

    nc = tc.nc
    f32 = mybir.dt.float32
    bf16 = mybir.dt.bfloat16
    FCS = [(0, 128), (128, 128), (256, 1)]   # f chunks for transposes
    SLOT_GROUPS = [(0, 4), (4, 4), (8, 2)]   # batches of slots per psum tile

    const = ctx.enter_context(tc.tile_pool(name="const", bufs=1))
    mcpool = ctx.enter_context(tc.tile_pool(name="mc", bufs=1))
    xpool = ctx.enter_context(tc.tile_pool(name="xstage", bufs=1))
    # One PSUM pool; the 8 banks rotate roles via tag reuse:
    # head ptg transposes -> accr/acci accumulators -> tail out collectors.
    psum = ctx.enter_context(tc.tile_pool(name="psum", bufs=1, space="PSUM"))
    BTAGS = [f"bank{i}" for i in range(8)]

    # ---- SWDGE casting-load order: tap 0's pair first (it gates the DVE's
    # first op), then identb (gates PE transposes), tap 0's single, then x,
    # then the remaining taps. ident f32 via HWDGE sync (slow startup fine:
    # only the tail's fp32 pair-transposes read it).
    mp = {}
    ms = {}

    def load_mpair(n):
        p = mcpool.tile([TP, 2 * MW], bf16, tag=f"mp{n}", name=f"mp{n}")
        nc.gpsimd.dma_start(
            p.rearrange("p (c w) -> p c w", c=2),
            m_ap[9 + n:19 + n:9].rearrange("c (p t) f -> p c (t f)", p=TP),
        )
        mp[n] = p

    def load_msingle(n):
        s = mcpool.tile([TP, MW], bf16, tag=f"ms{n}", name=f"ms{n}")
        nc.gpsimd.dma_start(s[:], m_ap[n].rearrange("(p t) f -> p (t f)", p=TP))
        ms[n] = s

    identb = const.tile([128, 128], bf16, tag="identb")
    nc.gpsimd.dma_start(identb[:], id_ap)

    ident = const.tile([128, 128], f32, tag="ident")
    nc.sync.dma_start(ident[:], id_ap)

    xns = []
    for ci, (f0, fw) in enumerate(FCS):
        xn = xpool.tile([fw, (T + 2) * 2], bf16, tag=f"xn{f0}", name=f"xn{f0}")
        nc.vector.memset(xn[:, 0:4], 0.0)
        nc.gpsimd.dma_start(
            xn[:, 4:], x_ap[f0:f0 + fw].rearrange("f t c -> f (t c)")
        )
        xns.append(xn)

    for n in range(9):
        load_mpair(n)
        load_msingle(n)

    # ---- x planes (bf16): f origin at col 1, zero pads at cols 0, 258, 259.
    # xr/xi are plain; xrs = +s*xr and xin = -s*xi carry the basis scale.
    planes = {}
    for nm in ("xr", "xi", "xrs", "xin"):
        p = const.tile([TP, PLW], bf16, tag=nm, name=nm)
        if nm in ("xr", "xi"):
            pv = p.rearrange("p (s w) -> p s w", w=SROW)
            nc.vector.memset(pv[:, :, 0:1], 0.0)
            nc.vector.memset(pv[:, :, 258:260], 0.0)
        planes[nm] = p

    # sideband (f=256) accumulators, one per complex component
    sbr = const.tile([TP, TAU], bf16, tag="sbr")
    sbi = const.tile([TP, TAU], bf16, tag="sbi")
    nc.vector.memset(sbr[:], 0.0)
    nc.vector.memset(sbi[:], 0.0)

    # ---- transpose x into the planes (fp32 PE transposes into psum banks).
    # Copies cast fp32->bf16: slot groups g0 on the idle DVE, g1/g2 on ACT.
    # Scaled planes built per group right after (DVE for g0, ACT for rest).
    bank_rr = 0
    for gi, (g0, gn) in enumerate(SLOT_GROUPS):
        for ci, (f0, fw) in enumerate(FCS):
            xn3 = xns[ci].rearrange("f (t c) -> f t c", c=2)
            for q, nm in enumerate(("xr", "xi")):
                pA = planes[nm].rearrange("p (s w) -> p s w", w=SROW)
                ptg = psum.tile(
                    [TP, 512], bf16, tag=BTAGS[bank_rr % 8], name="ptg",
                    padded_shape=[128, 1024],
                )
                bank_rr += 1
                for u in range(gn):
                    ts = g0 + u
                    nc.tensor.transpose(
                        ptg[0:TP, 128 * u:128 * u + fw],
                        xn3[0:fw, ts:ts + TAU * (TP - 1) + 1:TAU, q],
                        identb[0:fw, 0:fw],
                    )
                src = ptg.rearrange("p (u w) -> p u w", w=128)[0:TP, 0:gn, 0:fw]
                dst = pA[:, g0:g0 + gn, 1 + f0:1 + f0 + fw]
                if gi == 0:
                    nc.vector.tensor_copy(dst, src)
                else:
                    nc.scalar.copy(dst, src)
        # scaled planes for this slot group
        for src_nm, dst_nm, sc in (("xr", "xrs", SQ3H), ("xi", "xin", -SQ3H)):
            sv = planes[src_nm].rearrange("p (s w) -> p s w", w=SROW)
            dv = planes[dst_nm].rearrange("p (s w) -> p s w", w=SROW)
            if gi == 0:
                nc.vector.tensor_scalar_mul(
                    dv[:, g0:g0 + gn, :], sv[:, g0:g0 + gn, :], sc)
            else:
                nc.scalar.mul(dv[:, g0:g0 + gn, :], sv[:, g0:g0 + gn, :], sc)

    prep = ctx.enter_context(tc.tile_pool(name="prep", bufs=2))
    prod = ctx.enter_context(tc.tile_pool(name="prod", bufs=4))
    yop = ctx.enter_context(tc.tile_pool(name="yop", bufs=1))

    # ---- PSUM accumulators: accr rows of 256 in banks 0-3, acci in 4-7.
    # 512 fp32 = one bank = two tau rows.
    accr_c = [
        psum.tile([TP, 512], f32, tag=BTAGS[c], name=f"accr{c}",
                  padded_shape=[128, 512])
        for c in range(4)
    ]
    acci_c = [
        psum.tile([TP, 512], f32, tag=BTAGS[4 + c], name=f"acci{c}",
                  padded_shape=[128, 512])
        for c in range(4)
    ]

    idw = identb[0:TP, 0:TP]

    def prep_td(n, r0=0, rn=TAU, th_on_dve=False):
        """DVE half of prep: t1 = m9+m18, d = m9-m18 (tile-local rows).
        th = -0.5*t1 runs on ACT (one-tap lookahead) except when the ACT
        queue is backlogged (first tap, during the head copies)."""
        sl = slice(r0 * F, (r0 + rn) * F)
        m9 = mp[n][:, 0:MW][:, sl]
        m18 = mp[n][:, MW:2 * MW][:, sl]
        t1 = prep.tile([TP, rn * F], bf16, tag="t1", name="t1", bufs=1)
        nc.vector.tensor_add(t1[:], m9, m18)
        d = prep.tile([TP, rn * F], bf16, tag="d", name="d", bufs=1)
        nc.vector.tensor_sub(d[:], m9, m18)
        th = prep.tile([TP, rn * F], bf16, tag="th", name="th")
        if th_on_dve:
            nc.vector.tensor_scalar_mul(th[:], t1[:], -0.5)
        else:
            nc.scalar.mul(th[:], t1[:], -0.5)
        return th, d

    def prep_ar(n, th, r0=0, rn=TAU):
        ar = prep.tile([TP, rn * F], bf16, tag="ar", name="ar", bufs=1)
        nc.vector.tensor_add(ar[:], th[:], ms[n][:, r0 * F:(r0 + rn) * F])
        return ar

    def products(n, ar, d, r0, rn):
        """4 plain-mul product tiles for tau rows [r0, r0+rn) (tile-local)."""
        i, j = divmod(n, 3)

        def xv(nm):
            return planes[nm].rearrange("p (s w) -> p s w", w=SROW)[
                :, i + r0:i + r0 + rn, j:j + F]

        a8 = ar.rearrange("p (r w) -> p r w", w=F)
        d8 = d.rearrange("p (r w) -> p r w", w=F)
        w = rn * F
        p0 = prod.tile([TP, w], bf16, tag="P", name="p0")
        p1 = prod.tile([TP, w], bf16, tag="P", name="p1")
        p2 = prod.tile([TP, w], bf16, tag="P", name="p2")
        p3 = prod.tile([TP, w], bf16, tag="P", name="p3")
        nc.vector.tensor_mul(p0.rearrange("p (r w) -> p r w", w=F), a8, xv("xr"))
        nc.vector.tensor_mul(p1.rearrange("p (r w) -> p r w", w=F), a8, xv("xi"))
        nc.vector.tensor_mul(p2.rearrange("p (r w) -> p r w", w=F), d8, xv("xin"))
        nc.vector.tensor_mul(p3.rearrange("p (r w) -> p r w", w=F), d8, xv("xrs"))
        return p0, p1, p2, p3

    def accum_main(n, tiles, r0, rn, last):
        """PE-accumulate tau rows [r0, r0+rn) of the products into PSUM."""
        p0, p1, p2, p3 = tiles
        first = n == 0
        for c in range(r0 // 2, (r0 + rn) // 2):
            lo = c * 2 - r0
            for acc, pa, pb in ((accr_c[c], p0, p2), (acci_c[c], p1, p3)):
                for k, pt in enumerate((pa, pb)):
                    pv = pt.rearrange("p (r w) -> p r w", w=F)[
                        :, lo:lo + 2, 0:256]
                    nc.tensor.matmul(
                        acc[:], idw, pv,
                        start=(first and k == 0),
                        stop=(last and k == 1),
                    )

    def accum_sb(n, tiles, r0, rn):
        if n % 3 == 2:
            return
        p0, p1, p2, p3 = tiles
        for acc, pa, pb in ((sbr, p0, p2), (sbi, p1, p3)):
            for pt in (pa, pb):
                pv = pt.rearrange("p (r w) -> p r w", w=F)[:, 0:rn, 256]
                nc.vector.tensor_add(acc[:, r0:r0 + rn], acc[:, r0:r0 + rn], pv)

    # drained accumulators in (f-major, tau-minor) bf16 layout: adjacent tau
    # pairs (t=8p+2q, +1) are then adjacent bytes, so the output transposes
    # can run on fp32-reinterpreted PAIRS (psum matmul writes need 4B align).
    acc_s = [
        const.tile([TP, AW], bf16, tag="accr_s", name="accr_s"),
        const.tile([TP, AW], bf16, tag="acci_s", name="acci_s"),
    ]
    acc32 = [a.bitcast(f32) for a in acc_s]

    yo01 = yop.tile([128, 2 * T * 2], f32, tag="yo01", name="yo01")
    yviews = [
        yo01[:, 0:T * 2].rearrange("f (t c) -> f t c", c=2),
        yo01[:, T * 2:].rearrange("f (t c) -> f t c", c=2),
    ]

    # output collector psum banks: one per (f-half, comp); the 4 fp32 pair
    # transposes land strided (pair position = 4p+q fp32) into one bank,
    # leaving it t-contiguous bf16; then ONE copy into yo01.
    # Banks are reused in drain-retirement order (quarter c frees c and 4+c).
    OBANK = {(0, 0): 0, (0, 1): 4, (1, 0): 1, (1, 1): 5}
    obank = {}

    def drain_chunk(c):
        for comp, acc in ((0, accr_c[c]), (1, acci_c[c])):
            src = acc.rearrange("p (r f) -> p f r", r=2)
            dst = acc_s[comp].rearrange("p (f r) -> p f r", r=TAU)[
                :, :, 2 * c:2 * c + 2]
            nc.scalar.copy(dst, src)

    out_done = {k: 0 for k in OBANK}

    def out_rows(q):
        """After drain_chunk(q): transpose every tau PAIR that is both
        drained (pair <= q) and whose collector bank is retired (f0's
        banks 0/4 retire at q>=0, f1's banks 1/5 at q>=1)."""
        for ci in (0, 1):
            if q < ci:
                continue
            f0, fw = FCS[ci]
            for comp in (0, 1):
                key = (ci, comp)
                if key not in obank:
                    obank[key] = psum.tile(
                        [128, T // 2], f32, tag=BTAGS[OBANK[key]],
                        name=f"ob{ci}{comp}", padded_shape=[128, 512],
                    )
                ob = obank[key]
                a32 = acc32[comp].rearrange("p (f r) -> p f r", r=TAU // 2)
                for rp in range(out_done[key], q + 1):
                    # each strided transpose is its own single-matmul group
                    # (start=False accumulation into untouched elements is
                    # not safe; disjoint start=True writes are).
                    nc.tensor.matmul(
                        ob[0:fw, rp:rp + 4 * (TP - 1) + 1:4],
                        a32[:, f0:f0 + fw, rp],
                        ident[0:TP, 0:TP],
                        is_transpose=True,
                    )
                out_done[key] = q + 1

    for n in range(C // 3):
        if n == 0:
            th0, d0 = prep_td(0, th_on_dve=True)
            ar0, dd = prep_ar(0, th0), d0
        if n < 8:
            nxt_td = prep_td(n + 1)
            tiles = products(n, ar0, dd, 0, TAU)
            accum_main(n, tiles, 0, TAU, last=False)
            accum_sb(n, tiles, 0, TAU)
            ar0, dd = prep_ar(n + 1, nxt_td[0]), nxt_td[1]
        else:
            tiles = products(n, ar0, dd, 0, TAU)
            accum_main(n, tiles, 0, TAU, last=True)
            # drains + output transposes staged per retired psum chunk
            for q in range(4):
                drain_chunk(q)
                out_rows(q)

    # ---- one contiguous-psum-source copy per (f-half, comp), then stores.
    # DVE takes the real comps, ACT the imag; casting SWDGE stores per half.
    for ci in (0, 1):
        for comp in (0, 1):
            dst = yviews[ci][0:128, :, comp]
            src = obank[(ci, comp)].bitcast(bf16)[0:128, 0:T]
            if comp == 0:
                nc.vector.tensor_copy(dst, src)
            else:
                nc.scalar.copy(dst, src)
        nc.gpsimd.dma_start(
            y_ap[128 * ci:128 * (ci + 1)].rearrange("f t c -> f (t c)"),
            yo01[:, T * 2 * ci:T * 2 * (ci + 1)],
        )
    # f=256 sideband: interleave (t, c) as f32 in SBUF (2 tiny ACT copies),
    # then one contiguous-row store (64B runs per partition).
    sbri = yop.tile([TP, 2 * TAU], f32, tag="sbri", name="sbri")
    sbv = sbri.rearrange("p (t c) -> p t c", c=2)
    nc.scalar.copy(sbv[:, :, 0], sbr[:])
    nc.scalar.copy(sbv[:, :, 1], sbi[:])
    nc.gpsimd.dma_start(
        y_ap[256].rearrange("(p t) c -> p (t c)", p=TP), sbri[:]
    )


def _build():
    if "nc" in _CACHE:
        return _CACHE["nc"]
    from contextlib import ExitStack
    from concourse import bacc, mybir
    import concourse.tile as tile

    f32 = mybir.dt.float32
    nc = bacc.Bacc("TRN2", target_bir_lowering=False, debug=False, num_devices=B)
    m_d = nc.dram_tensor("m", (C, T, F), f32, kind="ExternalInput")
    x_d = nc.dram_tensor("x", (F, T, 2), f32, kind="ExternalInput")
    id_d = nc.dram_tensor("ident", (128, 128), f32, kind="ExternalInput")
    y_d = nc.dram_tensor("y", (F, T, 2), f32, kind="ExternalOutput")

    with tile.TileContext(nc) as tc:
        with ExitStack() as ctx:
            _emit(ctx, tc, m_d.ap(), x_d.ap(), id_d.ap(), y_d.ap())
    nc.compile()
    _CACHE["nc"] = nc
    return nc


def _in_maps(m, x):
    ident = np.eye(128, dtype=np.float32)
    return [
        {"m": np.ascontiguousarray(m[b]), "x": np.ascontiguousarray(x[b]),
         "ident": ident}
        for b in range(B)
    ]


def kernel(m, x, v, _trace=False):
    from concourse import bass_utils

    m = np.asarray(m, dtype=np.float32)
    x = np.asarray(x, dtype=np.float32)
    nc = _build()
    res = bass_utils.run_bass_kernel_spmd(
        nc, _in_maps(m, x), core_ids=list(range(B)), trace=_trace
    )
    kernel.last_results = res
    y = np.stack(
        [np.asarray(res.results[b]["y"], dtype=np.float32) for b in range(B)],
        axis=0,
    )
    return y


# revision 32
# speedup vs baseline: 1.1762x; 1.0106x over previous
"""Trainium2 Bass kernel for nn_CCM: per-pixel complex 3x3 conv mask.

Math (per batch element b, sharded 1 batch element per NeuronCore):
  y[t,f] = sum_{n=0..8} A_n[t,f] * X[t+i(n)-2, f+j(n)-1]   (complex)
with A_n = m_n + w * m_{9+n} + conj(w) * m_{18+n}, w = -1/2 + i*sqrt(3)/2:
  Ar_n = m_n - 0.5*(m_{9+n} + m_{18+n})
  Ai_n = s * (m_{9+n} - m_{18+n}),  s = sqrt(3)/2
X = xr + i*xi, zero padded (causal in t: 2 top; symmetric in f: 1,1).

v4 design (trace-driven, from v1-v3):
- The kernel is jointly limited by the 29.8MB fp32 load stream (~84us at
  358GB/s) and the DVE. Everything else is arranged to keep both saturated:
- Per-tap products accumulate into PSUM fp32 via identity-weight matmuls on
  the PE (removes all accumulation adds from the DVE; better precision than
  bf16 accumulators).
- The +-s basis scale lives in two extra SCALED x planes (xrs=+s*xr,
  xin=-s*xi); th=-0.5*(m9+m18) runs on ACT with one-tap lookahead so the
  DVE never waits. DVE per tap: t1, d, ar + 4 plain 2x tensor_tensor muls.
- x stages via HWDGE (sync) as fp32 so the SWDGE m-stream starts ~4us
  earlier; x transposes run in fp32, plane copies cast to bf16.
- PSUM holds 4096 fp32/partition but the output needs 2*2056 so the f=256
  column accumulates in SBUF via tiny strided DVE adds (j==2 taps are zero
  there and skipped); it stores via a direct scatter SWDGE cast DMA.
- Output: per (f-half, comp) ONE psum bank collects all 8 tau-row
  transposes (start=False accumulate-into-disjoint-elements), then ONE
  contiguous-source copy into the bf16 staging tile; casting SWDGE stores.
- Tap 8's m tiles load in tau-quarters so its products/dr ains/transposes
  pipeline with the last DMA arrivals.
- PSUM banks are time-shared via same-tag tile reuse: head x-transposes ->
  accumulators -> tail output collectors.
"""

import sys
import numpy as np

sys.path.insert(0, "/opt/trn_rl_repo")

B = 8
C = 27
T = 1000
F = 257
TP = 125          # partitions
TAU = 8           # t = 8*p + tau
NS = 10           # slots in x planes: tau in [-2, 8)
SROW = 260        # x plane slot row width (elements)
MW = TAU * F      # 2056: m / prod tile width (flat, rows of 257)
AW = TAU * 256    # 2048: psum accumulator width (rows of 256)
PLW = NS * SROW   # 2600: x plane width
QW = 2 * F        # 514: one tau-quarter of a flat m plane
SQ3H = float(np.sqrt(3.0) / 2.0)

_CACHE = {}


def _emit(ctx, tc, m_ap, x_ap, id_ap, y_ap):
    import concourse.mybir as mybir

    nc = tc.nc
    f32 = mybir.dt.float32
    bf16 = mybir.dt.bfloat16
    FCS = [(0, 128), (128, 128), (256, 1)]   # f chunks for transposes
    SLOT_GROUPS = [(0, 4), (4, 4), (8, 2)]   # batches of slots per psum tile

    const = ctx.enter_context(tc.tile_pool(name="const", bufs=1))
    mcpool = ctx.enter_context(tc.tile_pool(name="mc", bufs=1))
    xpool = ctx.enter_context(tc.tile_pool(name="xstage", bufs=1))
    # One PSUM pool; the 8 banks rotate roles via tag reuse:
    # head ptg transposes -> accr/acci accumulators -> tail out collectors.
    psum = ctx.enter_context(tc.tile_pool(name="psum", bufs=1, space="PSUM"))
    BTAGS = [f"bank{i}" for i in range(8)]

    # ---- SWDGE casting-load order: tap 0's pair first (it gates the DVE's
    # first op), then identb (gates PE transposes), tap 0's single, then x,
    # then the remaining taps. ident f32 via HWDGE sync (slow startup fine:
    # only the tail's fp32 pair-transposes read it).
    mp = {}
    ms = {}

    def load_mpair(n):
        # two sequential per-plane DMAs: a single c-strided pair DMA whose
        # descriptors alternate between HBM regions 9 planes apart measurably
        # drops SWDGE throughput (DRAM page locality)
        p = mcpool.tile([TP, 2 * MW], bf16, tag=f"mp{n}", name=f"mp{n}")
        nc.gpsimd.dma_start(
            p[:, 0:MW], m_ap[9 + n].rearrange("(p t) f -> p (t f)", p=TP)
        )
        nc.gpsimd.dma_start(
            p[:, MW:2 * MW], m_ap[18 + n].rearrange("(p t) f -> p (t f)", p=TP)
        )
        mp[n] = p

    def load_msingle(n):
        s = mcpool.tile([TP, MW], bf16, tag=f"ms{n}", name=f"ms{n}")
        nc.gpsimd.dma_start(s[:], m_ap[n].rearrange("(p t) f -> p (t f)", p=TP))
        ms[n] = s

    identb = const.tile([128, 128], bf16, tag="identb")
    nc.gpsimd.dma_start(identb[:], id_ap)

    ident = const.tile([128, 128], f32, tag="ident")
    nc.sync.dma_start(ident[:], id_ap)

    xns = []
    for ci, (f0, fw) in enumerate(FCS):
        xn = xpool.tile([fw, (T + 2) * 2], bf16, tag=f"xn{f0}", name=f"xn{f0}")
        nc.vector.memset(xn[:, 0:4], 0.0)
        nc.gpsimd.dma_start(
            xn[:, 4:], x_ap[f0:f0 + fw].rearrange("f t c -> f (t c)")
        )
        xns.append(xn)

    for n in range(9):
        load_mpair(n)
        load_msingle(n)

    # ---- x planes (bf16): f origin at col 1, zero pads at cols 0, 258, 259.
    # xr/xi are plain; xrs = +s*xr and xin = -s*xi carry the basis scale.
    planes = {}
    for nm in ("xr", "xi", "xrs", "xin"):
        p = const.tile([TP, PLW], bf16, tag=nm, name=nm)
        if nm in ("xr", "xi"):
            pv = p.rearrange("p (s w) -> p s w", w=SROW)
            nc.vector.memset(pv[:, :, 0:1], 0.0)
            nc.vector.memset(pv[:, :, 258:260], 0.0)
        planes[nm] = p

    # sideband (f=256) accumulators, one per complex component
    sbr = const.tile([TP, TAU], bf16, tag="sbr")
    sbi = const.tile([TP, TAU], bf16, tag="sbi")
    nc.vector.memset(sbr[:], 0.0)
    nc.vector.memset(sbi[:], 0.0)

    # ---- transpose x into the planes (fp32 PE transposes into psum banks).
    # Copies cast fp32->bf16: slot groups g0 on the idle DVE, g1/g2 on ACT.
    # Scaled planes built per group right after (DVE for g0, ACT for rest).
    bank_rr = 0
    for gi, (g0, gn) in enumerate(SLOT_GROUPS):
        for ci, (f0, fw) in enumerate(FCS):
            xn3 = xns[ci].rearrange("f (t c) -> f t c", c=2)
            for q, nm in enumerate(("xr", "xi")):
                pA = planes[nm].rearrange("p (s w) -> p s w", w=SROW)
                ptg = psum.tile(
                    [TP, 512], bf16, tag=BTAGS[bank_rr % 8], name="ptg",
                    padded_shape=[128, 1024],
                )
                bank_rr += 1
                for u in range(gn):
                    ts = g0 + u
                    nc.tensor.transpose(
                        ptg[0:TP, 128 * u:128 * u + fw],
                        xn3[0:fw, ts:ts + TAU * (TP - 1) + 1:TAU, q],
                        identb[0:fw, 0:fw],
                    )
                src = ptg.rearrange("p (u w) -> p u w", w=128)[0:TP, 0:gn, 0:fw]
                dst = pA[:, g0:g0 + gn, 1 + f0:1 + f0 + fw]
                if gi == 0:
                    nc.vector.tensor_copy(dst, src)
                else:
                    nc.scalar.copy(dst, src)
        # scaled planes for this slot group
        for src_nm, dst_nm, sc in (("xr", "xrs", SQ3H), ("xi", "xin", -SQ3H)):
            sv = planes[src_nm].rearrange("p (s w) -> p s w", w=SROW)
            dv = planes[dst_nm].rearrange("p (s w) -> p s w", w=SROW)
            if gi == 0:
                nc.vector.tensor_scalar_mul(
                    dv[:, g0:g0 + gn, :], sv[:, g0:g0 + gn, :], sc)
            else:
                nc.scalar.mul(dv[:, g0:g0 + gn, :], sv[:, g0:g0 + gn, :], sc)

    prep = ctx.enter_context(tc.tile_pool(name="prep", bufs=2))
    prod = ctx.enter_context(tc.tile_pool(name="prod", bufs=4))
    yop = ctx.enter_context(tc.tile_pool(name="yop", bufs=1))

    # ---- PSUM accumulators: accr rows of 256 in banks 0-3, acci in 4-7.
    # 512 fp32 = one bank = two tau rows.
    accr_c = [
        psum.tile([TP, 512], f32, tag=BTAGS[c], name=f"accr{c}",
                  padded_shape=[128, 512])
        for c in range(4)
    ]
    acci_c = [
        psum.tile([TP, 512], f32, tag=BTAGS[4 + c], name=f"acci{c}",
                  padded_shape=[128, 512])
        for c in range(4)
    ]

    idw = identb[0:TP, 0:TP]

    def prep_td(n, r0=0, rn=TAU, th_on_dve=False):
        """DVE half of prep: t1 = m9+m18, d = m9-m18 (tile-local rows).
        th = -0.5*t1 runs on ACT (one-tap lookahead) except when the ACT
        queue is backlogged (first tap, during the head copies)."""
        sl = slice(r0 * F, (r0 + rn) * F)
        m9 = mp[n][:, 0:MW][:, sl]
        m18 = mp[n][:, MW:2 * MW][:, sl]
        t1 = prep.tile([TP, rn * F], bf16, tag="t1", name="t1", bufs=1)
        nc.vector.tensor_add(t1[:], m9, m18)
        d = prep.tile([TP, rn * F], bf16, tag="d", name="d", bufs=1)
        nc.vector.tensor_sub(d[:], m9, m18)
        th = prep.tile([TP, rn * F], bf16, tag="th", name="th")
        if th_on_dve:
            nc.vector.tensor_scalar_mul(th[:], t1[:], -0.5)
        else:
            nc.scalar.mul(th[:], t1[:], -0.5)
        return th, d

    def prep_ar(n, th, r0=0, rn=TAU):
        ar = prep.tile([TP, rn * F], bf16, tag="ar", name="ar", bufs=1)
        nc.vector.tensor_add(ar[:], th[:], ms[n][:, r0 * F:(r0 + rn) * F])
        return ar

    def products(n, ar, d, r0, rn):
        """4 plain-mul product tiles for tau rows [r0, r0+rn) (tile-local)."""
        i, j = divmod(n, 3)

        def xv(nm):
            return planes[nm].rearrange("p (s w) -> p s w", w=SROW)[
                :, i + r0:i + r0 + rn, j:j + F]

        a8 = ar.rearrange("p (r w) -> p r w", w=F)
        d8 = d.rearrange("p (r w) -> p r w", w=F)
        w = rn * F
        p0 = prod.tile([TP, w], bf16, tag="P", name="p0")
        p1 = prod.tile([TP, w], bf16, tag="P", name="p1")
        p2 = prod.tile([TP, w], bf16, tag="P", name="p2")
        p3 = prod.tile([TP, w], bf16, tag="P", name="p3")
        nc.vector.tensor_mul(p0.rearrange("p (r w) -> p r w", w=F), a8, xv("xr"))
        nc.vector.tensor_mul(p1.rearrange("p (r w) -> p r w", w=F), a8, xv("xi"))
        nc.vector.tensor_mul(p2.rearrange("p (r w) -> p r w", w=F), d8, xv("xin"))
        nc.vector.tensor_mul(p3.rearrange("p (r w) -> p r w", w=F), d8, xv("xrs"))
        return p0, p1, p2, p3

    def accum_main(n, tiles, r0, rn, last):
        """PE-accumulate tau rows [r0, r0+rn) of the products into PSUM."""
        p0, p1, p2, p3 = tiles
        first = n == 0
        for c in range(r0 // 2, (r0 + rn) // 2):
            lo = c * 2 - r0
            for acc, pa, pb in ((accr_c[c], p0, p2), (acci_c[c], p1, p3)):
                for k, pt in enumerate((pa, pb)):
                    pv = pt.rearrange("p (r w) -> p r w", w=F)[
                        :, lo:lo + 2, 0:256]
                    nc.tensor.matmul(
                        acc[:], idw, pv,
                        start=(first and k == 0),
                        stop=(last and k == 1),
                    )

    def accum_sb(n, tiles, r0, rn):
        if n % 3 == 2:
            return
        p0, p1, p2, p3 = tiles
        for acc, pa, pb in ((sbr, p0, p2), (sbi, p1, p3)):
            for pt in (pa, pb):
                pv = pt.rearrange("p (r w) -> p r w", w=F)[:, 0:rn, 256]
                nc.vector.tensor_add(acc[:, r0:r0 + rn], acc[:, r0:r0 + rn], pv)

    # drained accumulators in (f-major, tau-minor) bf16 layout: adjacent tau
    # pairs (t=8p+2q, +1) are then adjacent bytes, so the output transposes
    # can run on fp32-reinterpreted PAIRS (psum matmul writes need 4B align).
    acc_s = [
        const.tile([TP, AW], bf16, tag="accr_s", name="accr_s"),
        const.tile([TP, AW], bf16, tag="acci_s", name="acci_s"),
    ]
    acc32 = [a.bitcast(f32) for a in acc_s]

    yo01 = yop.tile([128, 2 * T * 2], f32, tag="yo01", name="yo01")
    yviews = [
        yo01[:, 0:T * 2].rearrange("f (t c) -> f t c", c=2),
        yo01[:, T * 2:].rearrange("f (t c) -> f t c", c=2),
    ]

    # output collector psum banks: one per (f-half, comp); the 4 fp32 pair
    # transposes land strided (pair position = 4p+q fp32) into one bank,
    # leaving it t-contiguous bf16; then ONE copy into yo01.
    # Banks are reused in drain-retirement order (quarter c frees c and 4+c).
    OBANK = {(0, 0): 0, (0, 1): 4, (1, 0): 1, (1, 1): 5}
    obank = {}

    def drain_chunk(c):
        for comp, acc in ((0, accr_c[c]), (1, acci_c[c])):
            src = acc.rearrange("p (r f) -> p f r", r=2)
            dst = acc_s[comp].rearrange("p (f r) -> p f r", r=TAU)[
                :, :, 2 * c:2 * c + 2]
            nc.scalar.copy(dst, src)

    out_done = {k: 0 for k in OBANK}

    def out_rows(q):
        """After drain_chunk(q): transpose every tau PAIR that is both
        drained (pair <= q) and whose collector bank is retired (f0's
        banks 0/4 retire at q>=0, f1's banks 1/5 at q>=1)."""
        for ci in (0, 1):
            if q < ci:
                continue
            f0, fw = FCS[ci]
            for comp in (0, 1):
                key = (ci, comp)
                if key not in obank:
                    obank[key] = psum.tile(
                        [128, T // 2], f32, tag=BTAGS[OBANK[key]],
                        name=f"ob{ci}{comp}", padded_shape=[128, 512],
                    )
                ob = obank[key]
                a32 = acc32[comp].rearrange("p (f r) -> p f r", r=TAU // 2)
                for rp in range(out_done[key], q + 1):
                    # each strided transpose is its own single-matmul group
                    # (start=False accumulation into untouched elements is
                    # not safe; disjoint start=True writes are).
                    nc.tensor.matmul(
                        ob[0:fw, rp:rp + 4 * (TP - 1) + 1:4],
                        a32[:, f0:f0 + fw, rp],
                        ident[0:TP, 0:TP],
                        is_transpose=True,
                    )
                out_done[key] = q + 1

    for n in range(C // 3):
        if n == 0:
            th0, d0 = prep_td(0, th_on_dve=True)
            ar0, dd = prep_ar(0, th0), d0
        if n < 8:
            nxt_td = prep_td(n + 1)
            tiles = products(n, ar0, dd, 0, TAU)
            accum_main(n, tiles, 0, TAU, last=False)
            accum_sb(n, tiles, 0, TAU)
            ar0, dd = prep_ar(n + 1, nxt_td[0]), nxt_td[1]
        else:
            tiles = products(n, ar0, dd, 0, TAU)
            accum_main(n, tiles, 0, TAU, last=True)
            # drains + output transposes staged per retired psum chunk
            for q in range(4):
                drain_chunk(q)
                out_rows(q)

    # ---- one contiguous-psum-source copy per (f-half, comp), then stores.
    # DVE takes the real comps, ACT the imag; casting SWDGE stores per half.
    for ci in (0, 1):
        for comp in (0, 1):
            dst = yviews[ci][0:128, :, comp]
            src = obank[(ci, comp)].bitcast(bf16)[0:128, 0:T]
            if comp == 0:
                nc.vector.tensor_copy(dst, src)
            else:
                nc.scalar.copy(dst, src)
        nc.gpsimd.dma_start(
            y_ap[128 * ci:128 * (ci + 1)].rearrange("f t c -> f (t c)"),
            yo01[:, T * 2 * ci:T * 2 * (ci + 1)],
        )
    # f=256 sideband: interleave (t, c) as f32 in SBUF (2 tiny ACT copies),
    # then one contiguous-row store (64B runs per partition).
    sbri = yop.tile([TP, 2 * TAU], f32, tag="sbri", name="sbri")
    sbv = sbri.rearrange("p (t c) -> p t c", c=2)
    nc.scalar.copy(sbv[:, :, 0], sbr[:])
    nc.scalar.copy(sbv[:, :, 1], sbi[:])
    nc.gpsimd.dma_start(
        y_ap[256].rearrange("(p t) c -> p (t c)", p=TP), sbri[:]
    )


def _build():
    if "nc" in _CACHE:
        return _CACHE["nc"]
    from contextlib import ExitStack
    from concourse import bacc, mybir
    import concourse.tile as tile

    f32 = mybir.dt.float32
    nc = bacc.Bacc("TRN2", target_bir_lowering=False, debug=False, num_devices=B)
    m_d = nc.dram_tensor("m", (C, T, F), f32, kind="ExternalInput")
    x_d = nc.dram_tensor("x", (F, T, 2), f32, kind="ExternalInput")
    id_d = nc.dram_tensor("ident", (128, 128), f32, kind="ExternalInput")
    y_d = nc.dram_tensor("y", (F, T, 2), f32, kind="ExternalOutput")

    with tile.TileContext(nc) as tc:
        with ExitStack() as ctx:
            _emit(ctx, tc, m_d.ap(), x_d.ap(), id_d.ap(), y_d.ap())
    nc.compile()
    _CACHE["nc"] = nc
    return nc


def _in_maps(m, x):
    ident = np.eye(128, dtype=np.float32)
    return [
        {"m": np.ascontiguousarray(m[b]), "x": np.ascontiguousarray(x[b]),
         "ident": ident}
        for b in range(B)
    ]


def kernel(m, x, v, _trace=False):
    from concourse import bass_utils

    m = np.asarray(m, dtype=np.float32)
    x = np.asarray(x, dtype=np.float32)
    nc = _build()
    res = bass_utils.run_bass_kernel_spmd(
        nc, _in_maps(m, x), core_ids=list(range(B)), trace=_trace
    )
    kernel.last_results = res
    y = np.stack(
        [np.asarray(res.results[b]["y"], dtype=np.float32) for b in range(B)],
        axis=0,
    )
    return y


# revision 34
# speedup vs baseline: 1.2428x; 1.0566x over previous
"""Trainium2 Bass kernel for nn_CCM: per-pixel complex 3x3 conv mask.

Math (per batch element b, sharded 1 batch element per NeuronCore):
  y[t,f] = sum_{n=0..8} A_n[t,f] * X[t+i(n)-2, f+j(n)-1]   (complex)
with A_n = m_n + w * m_{9+n} + conj(w) * m_{18+n}, w = -1/2 + i*sqrt(3)/2:
  Ar_n = m_n - 0.5*(m_{9+n} + m_{18+n})
  Ai_n = s * (m_{9+n} - m_{18+n}),  s = sqrt(3)/2
X = xr + i*xi, zero padded (causal in t: 2 top; symmetric in f: 1,1).

v4 design (trace-driven, from v1-v3):
- The kernel is jointly limited by the 29.8MB fp32 load stream (~84us at
  358GB/s) and the DVE. Everything else is arranged to keep both saturated:
- Per-tap products accumulate into PSUM fp32 via identity-weight matmuls on
  the PE (removes all accumulation adds from the DVE; better precision than
  bf16 accumulators).
- The +-s basis scale lives in two extra SCALED x planes (xrs=+s*xr,
  xin=-s*xi); th=-0.5*(m9+m18) runs on ACT with one-tap lookahead so the
  DVE never waits. DVE per tap: t1, d, ar + 4 plain 2x tensor_tensor muls.
- x stages via HWDGE (sync) as fp32 so the SWDGE m-stream starts ~4us
  earlier; x transposes run in fp32, plane copies cast to bf16.
- PSUM holds 4096 fp32/partition but the output needs 2*2056 so the f=256
  column accumulates in SBUF via tiny strided DVE adds (j==2 taps are zero
  there and skipped); it stores via a direct scatter SWDGE cast DMA.
- Output: per (f-half, comp) ONE psum bank collects all 8 tau-row
  transposes (start=False accumulate-into-disjoint-elements), then ONE
  contiguous-source copy into the bf16 staging tile; casting SWDGE stores.
- Tap 8's m tiles load in tau-quarters so its products/dr ains/transposes
  pipeline with the last DMA arrivals.
- PSUM banks are time-shared via same-tag tile reuse: head x-transposes ->
  accumulators -> tail output collectors.
"""

import sys
import numpy as np

sys.path.insert(0, "/opt/trn_rl_repo")

B = 8
C = 27
T = 1000
F = 257
TP = 125          # partitions
TAU = 8           # t = 8*p + tau
NS = 10           # slots in x planes: tau in [-2, 8)
SROW = 260        # x plane slot row width (elements)
MW = TAU * F      # 2056: m / prod tile width (flat, rows of 257)
AW = TAU * 256    # 2048: psum accumulator width (rows of 256)
PLW = NS * SROW   # 2600: x plane width
QW = 2 * F        # 514: one tau-quarter of a flat m plane
SQ3H = float(np.sqrt(3.0) / 2.0)

_CACHE = {}


def _emit(ctx, tc, m_ap, x_ap, id_ap, y_ap):
    import concourse.mybir as mybir

    nc = tc.nc
    f32 = mybir.dt.float32
    bf16 = mybir.dt.bfloat16
    FCS = [(0, 128), (128, 128), (256, 1)]   # f chunks for transposes
    SLOT_GROUPS = [(0, 4), (4, 4), (8, 2)]   # batches of slots per psum tile

    const = ctx.enter_context(tc.tile_pool(name="const", bufs=1))
    mcpool = ctx.enter_context(tc.tile_pool(name="mc", bufs=1))
    xpool = ctx.enter_context(tc.tile_pool(name="xstage", bufs=1))
    # One PSUM pool; the 8 banks rotate roles via tag reuse:
    # head ptg transposes -> accr/acci accumulators -> tail out collectors.
    psum = ctx.enter_context(tc.tile_pool(name="psum", bufs=1, space="PSUM"))
    BTAGS = [f"bank{i}" for i in range(8)]

    # ---- SWDGE casting-load order: tap 0's pair first (it gates the DVE's
    # first op), then identb (gates PE transposes), tap 0's single, then x,
    # then the remaining taps. ident f32 via HWDGE sync (slow startup fine:
    # only the tail's fp32 pair-transposes read it).
    mp = {}
    ms = {}

    def load_mpair(n):
        # two sequential per-plane DMAs: a single c-strided pair DMA whose
        # descriptors alternate between HBM regions 9 planes apart measurably
        # drops SWDGE throughput (DRAM page locality)
        p = mcpool.tile([TP, 2 * MW], bf16, tag=f"mp{n}", name=f"mp{n}")
        nc.gpsimd.dma_start(
            p[:, 0:MW], m_ap[9 + n].rearrange("(p t) f -> p (t f)", p=TP)
        )
        nc.gpsimd.dma_start(
            p[:, MW:2 * MW], m_ap[18 + n].rearrange("(p t) f -> p (t f)", p=TP)
        )
        mp[n] = p

    def load_msingle(n):
        s = mcpool.tile([TP, MW], bf16, tag=f"ms{n}", name=f"ms{n}")
        nc.gpsimd.dma_start(s[:], m_ap[n].rearrange("(p t) f -> p (t f)", p=TP))
        ms[n] = s

    load_mpair(0)
    load_msingle(0)

    identb = const.tile([128, 128], bf16, tag="identb")
    nc.gpsimd.dma_start(identb[:], id_ap)

    ident = const.tile([128, 128], f32, tag="ident")
    nc.sync.dma_start(ident[:], id_ap)

    xns = []
    for ci, (f0, fw) in enumerate(FCS):
        xn = xpool.tile([fw, (T + 2) * 2], bf16, tag=f"xn{f0}", name=f"xn{f0}")
        nc.vector.memset(xn[:, 0:4], 0.0)
        nc.gpsimd.dma_start(
            xn[:, 4:], x_ap[f0:f0 + fw].rearrange("f t c -> f (t c)")
        )
        xns.append(xn)

    for n in range(1, 9):
        load_mpair(n)
        load_msingle(n)

    # ---- x planes (bf16): f origin at col 1, zero pads at cols 0, 258, 259.
    # xr/xi are plain; xrs = +s*xr and xin = -s*xi carry the basis scale.
    planes = {}
    for nm in ("xr", "xi", "xrs", "xin"):
        p = const.tile([TP, PLW], bf16, tag=nm, name=nm)
        if nm in ("xr", "xi"):
            pv = p.rearrange("p (s w) -> p s w", w=SROW)
            nc.vector.memset(pv[:, :, 0:1], 0.0)
            nc.vector.memset(pv[:, :, 258:260], 0.0)
        planes[nm] = p

    # sideband (f=256) accumulators, one per complex component
    sbr = const.tile([TP, TAU], bf16, tag="sbr")
    sbi = const.tile([TP, TAU], bf16, tag="sbi")
    nc.vector.memset(sbr[:], 0.0)
    nc.vector.memset(sbi[:], 0.0)

    # prep for tap 0 sits ahead of the head copies in the DVE queue: its
    # ops wait only on the m0 DMAs, measuring/overlapping the copy phase.
    prep = ctx.enter_context(tc.tile_pool(name="prep", bufs=2))

    def prep_td(n, r0=0, rn=TAU, th_on_dve=False):
        sl = slice(r0 * F, (r0 + rn) * F)
        m9 = mp[n][:, 0:MW][:, sl]
        m18 = mp[n][:, MW:2 * MW][:, sl]
        t1 = prep.tile([TP, rn * F], bf16, tag="t1", name="t1", bufs=1)
        nc.vector.tensor_add(t1[:], m9, m18)
        d = prep.tile([TP, rn * F], bf16, tag="d", name="d", bufs=1)
        nc.vector.tensor_sub(d[:], m9, m18)
        th = prep.tile([TP, rn * F], bf16, tag="th", name="th")
        if th_on_dve:
            nc.vector.tensor_scalar_mul(th[:], t1[:], -0.5)
        else:
            nc.scalar.mul(th[:], t1[:], -0.5)
        return th, d

    def prep_ar(n, th, r0=0, rn=TAU):
        ar = prep.tile([TP, rn * F], bf16, tag="ar", name="ar", bufs=1)
        nc.vector.tensor_add(ar[:], th[:], ms[n][:, r0 * F:(r0 + rn) * F])
        return ar

    th0_e, d0_e = prep_td(0, th_on_dve=True)
    ar0_e = prep_ar(0, th0_e)

    # ---- transpose x into the planes (fp32 PE transposes into psum banks).
    # Copies cast fp32->bf16: slot groups g0 on the idle DVE, g1/g2 on ACT.
    # Scaled planes built per group right after (DVE for g0, ACT for rest).
    bank_rr = 0
    for gi, (g0, gn) in enumerate(SLOT_GROUPS):
        for ci, (f0, fw) in enumerate(FCS):
            xn3 = xns[ci].rearrange("f (t c) -> f t c", c=2)
            for q, nm in enumerate(("xr", "xi")):
                pA = planes[nm].rearrange("p (s w) -> p s w", w=SROW)
                ptg = psum.tile(
                    [TP, 512], bf16, tag=BTAGS[bank_rr % 8], name="ptg",
                    padded_shape=[128, 1024],
                )
                bank_rr += 1
                for u in range(gn):
                    ts = g0 + u
                    nc.tensor.transpose(
                        ptg[0:TP, 128 * u:128 * u + fw],
                        xn3[0:fw, ts:ts + TAU * (TP - 1) + 1:TAU, q],
                        identb[0:fw, 0:fw],
                    )
                src = ptg.rearrange("p (u w) -> p u w", w=128)[0:TP, 0:gn, 0:fw]
                dst = pA[:, g0:g0 + gn, 1 + f0:1 + f0 + fw]
                if gi == 0:
                    nc.vector.tensor_copy(dst, src)
                else:
                    nc.scalar.copy(dst, src)
        # scaled planes for this slot group
        for src_nm, dst_nm, sc in (("xr", "xrs", SQ3H), ("xi", "xin", -SQ3H)):
            sv = planes[src_nm].rearrange("p (s w) -> p s w", w=SROW)
            dv = planes[dst_nm].rearrange("p (s w) -> p s w", w=SROW)
            if gi == 0:
                nc.vector.tensor_scalar_mul(
                    dv[:, g0:g0 + gn, :], sv[:, g0:g0 + gn, :], sc)
            else:
                nc.scalar.mul(dv[:, g0:g0 + gn, :], sv[:, g0:g0 + gn, :], sc)

    prod = ctx.enter_context(tc.tile_pool(name="prod", bufs=4))
    yop = ctx.enter_context(tc.tile_pool(name="yop", bufs=1))

    # ---- PSUM accumulators: accr rows of 256 in banks 0-3, acci in 4-7.
    # 512 fp32 = one bank = two tau rows.
    accr_c = [
        psum.tile([TP, 512], f32, tag=BTAGS[c], name=f"accr{c}",
                  padded_shape=[128, 512])
        for c in range(4)
    ]
    acci_c = [
        psum.tile([TP, 512], f32, tag=BTAGS[4 + c], name=f"acci{c}",
                  padded_shape=[128, 512])
        for c in range(4)
    ]

    idw = identb[0:TP, 0:TP]

    def products(n, ar, d, r0, rn):
        """4 plain-mul product tiles for tau rows [r0, r0+rn) (tile-local)."""
        i, j = divmod(n, 3)

        def xv(nm):
            return planes[nm].rearrange("p (s w) -> p s w", w=SROW)[
                :, i + r0:i + r0 + rn, j:j + F]

        a8 = ar.rearrange("p (r w) -> p r w", w=F)
        d8 = d.rearrange("p (r w) -> p r w", w=F)
        w = rn * F
        p0 = prod.tile([TP, w], bf16, tag="P", name="p0")
        p1 = prod.tile([TP, w], bf16, tag="P", name="p1")
        p2 = prod.tile([TP, w], bf16, tag="P", name="p2")
        p3 = prod.tile([TP, w], bf16, tag="P", name="p3")
        nc.vector.tensor_mul(p0.rearrange("p (r w) -> p r w", w=F), a8, xv("xr"))
        nc.vector.tensor_mul(p1.rearrange("p (r w) -> p r w", w=F), a8, xv("xi"))
        nc.vector.tensor_mul(p2.rearrange("p (r w) -> p r w", w=F), d8, xv("xin"))
        nc.vector.tensor_mul(p3.rearrange("p (r w) -> p r w", w=F), d8, xv("xrs"))
        return p0, p1, p2, p3

    def accum_main(n, tiles, r0, rn, last):
        """PE-accumulate tau rows [r0, r0+rn) of the products into PSUM."""
        p0, p1, p2, p3 = tiles
        first = n == 0
        for c in range(r0 // 2, (r0 + rn) // 2):
            lo = c * 2 - r0
            for acc, pa, pb in ((accr_c[c], p0, p2), (acci_c[c], p1, p3)):
                for k, pt in enumerate((pa, pb)):
                    pv = pt.rearrange("p (r w) -> p r w", w=F)[
                        :, lo:lo + 2, 0:256]
                    nc.tensor.matmul(
                        acc[:], idw, pv,
                        start=(first and k == 0),
                        stop=(last and k == 1),
                    )

    def accum_sb(n, tiles, r0, rn):
        if n % 3 == 2:
            return
        p0, p1, p2, p3 = tiles
        for acc, pa, pb in ((sbr, p0, p2), (sbi, p1, p3)):
            for pt in (pa, pb):
                pv = pt.rearrange("p (r w) -> p r w", w=F)[:, 0:rn, 256]
                nc.vector.tensor_add(acc[:, r0:r0 + rn], acc[:, r0:r0 + rn], pv)

    # drained accumulators in (f-major, tau-minor) bf16 layout: adjacent tau
    # pairs (t=8p+2q, +1) are then adjacent bytes, so the output transposes
    # can run on fp32-reinterpreted PAIRS (psum matmul writes need 4B align).
    acc_s = [
        const.tile([TP, AW], bf16, tag="accr_s", name="accr_s"),
        const.tile([TP, AW], bf16, tag="acci_s", name="acci_s"),
    ]
    acc32 = [a.bitcast(f32) for a in acc_s]

    yo01 = yop.tile([128, 2 * T * 2], f32, tag="yo01", name="yo01")
    yviews = [
        yo01[:, 0:T * 2].rearrange("f (t c) -> f t c", c=2),
        yo01[:, T * 2:].rearrange("f (t c) -> f t c", c=2),
    ]

    # output collector psum banks: one per (f-half, comp); the 4 fp32 pair
    # transposes land strided (pair position = 4p+q fp32) into one bank,
    # leaving it t-contiguous bf16; then ONE copy into yo01.
    # Banks are reused in drain-retirement order (quarter c frees c and 4+c).
    OBANK = {(0, 0): 0, (0, 1): 4, (1, 0): 1, (1, 1): 5}
    obank = {}

    def drain_chunk(c):
        for comp, acc in ((0, accr_c[c]), (1, acci_c[c])):
            src = acc.rearrange("p (r f) -> p f r", r=2)
            dst = acc_s[comp].rearrange("p (f r) -> p f r", r=TAU)[
                :, :, 2 * c:2 * c + 2]
            nc.scalar.copy(dst, src)

    out_done = {k: 0 for k in OBANK}

    def out_rows(q):
        """After drain_chunk(q): transpose every tau PAIR that is both
        drained (pair <= q) and whose collector bank is retired (f0's
        banks 0/4 retire at q>=0, f1's banks 1/5 at q>=1)."""
        for ci in (0, 1):
            if q < ci:
                continue
            f0, fw = FCS[ci]
            for comp in (0, 1):
                key = (ci, comp)
                if key not in obank:
                    obank[key] = psum.tile(
                        [128, T // 2], f32, tag=BTAGS[OBANK[key]],
                        name=f"ob{ci}{comp}", padded_shape=[128, 512],
                    )
                ob = obank[key]
                a32 = acc32[comp].rearrange("p (f r) -> p f r", r=TAU // 2)
                for rp in range(out_done[key], q + 1):
                    # each strided transpose is its own single-matmul group
                    # (start=False accumulation into untouched elements is
                    # not safe; disjoint start=True writes are).
                    nc.tensor.matmul(
                        ob[0:fw, rp:rp + 4 * (TP - 1) + 1:4],
                        a32[:, f0:f0 + fw, rp],
                        ident[0:TP, 0:TP],
                        is_transpose=True,
                    )
                out_done[key] = q + 1

    for n in range(C // 3):
        if n == 0:
            ar0, dd = ar0_e, d0_e
        if n < 8:
            nxt_td = prep_td(n + 1)
            tiles = products(n, ar0, dd, 0, TAU)
            accum_main(n, tiles, 0, TAU, last=False)
            accum_sb(n, tiles, 0, TAU)
            ar0, dd = prep_ar(n + 1, nxt_td[0]), nxt_td[1]
        else:
            tiles = products(n, ar0, dd, 0, TAU)
            accum_main(n, tiles, 0, TAU, last=True)
            # drains + output transposes staged per retired psum chunk
            for q in range(4):
                drain_chunk(q)
                out_rows(q)

    # ---- one contiguous-psum-source copy per (f-half, comp), then stores.
    # DVE takes the real comps, ACT the imag; casting SWDGE stores per half.
    for ci in (0, 1):
        for comp in (0, 1):
            dst = yviews[ci][0:128, :, comp]
            src = obank[(ci, comp)].bitcast(bf16)[0:128, 0:T]
            if comp == 0:
                nc.vector.tensor_copy(dst, src)
            else:
                nc.scalar.copy(dst, src)
        nc.gpsimd.dma_start(
            y_ap[128 * ci:128 * (ci + 1)].rearrange("f t c -> f (t c)"),
            yo01[:, T * 2 * ci:T * 2 * (ci + 1)],
        )
    # f=256 sideband: interleave (t, c) as f32 in SBUF (2 tiny ACT copies),
    # then one contiguous-row store (64B runs per partition).
    sbri = yop.tile([TP, 2 * TAU], f32, tag="sbri", name="sbri")
    sbv = sbri.rearrange("p (t c) -> p t c", c=2)
    nc.scalar.copy(sbv[:, :, 0], sbr[:])
    nc.scalar.copy(sbv[:, :, 1], sbi[:])
    nc.gpsimd.dma_start(
        y_ap[256].rearrange("(p t) c -> p (t c)", p=TP), sbri[:]
    )


def _build():
    if "nc" in _CACHE:
        return _CACHE["nc"]
    from contextlib import ExitStack
    from concourse import bacc, mybir
    import concourse.tile as tile

    f32 = mybir.dt.float32
    nc = bacc.Bacc("TRN2", target_bir_lowering=False, debug=False, num_devices=B)
    m_d = nc.dram_tensor("m", (C, T, F), f32, kind="ExternalInput")
    x_d = nc.dram_tensor("x", (F, T, 2), f32, kind="ExternalInput")
    id_d = nc.dram_tensor("ident", (128, 128), f32, kind="ExternalInput")
    y_d = nc.dram_tensor("y", (F, T, 2), f32, kind="ExternalOutput")

    with tile.TileContext(nc) as tc:
        with ExitStack() as ctx:
            _emit(ctx, tc, m_d.ap(), x_d.ap(), id_d.ap(), y_d.ap())
    nc.compile()
    _CACHE["nc"] = nc
    return nc


def _in_maps(m, x):
    ident = np.eye(128, dtype=np.float32)
    return [
        {"m": np.ascontiguousarray(m[b]), "x": np.ascontiguousarray(x[b]),
         "ident": ident}
        for b in range(B)
    ]


def kernel(m, x, v, _trace=False):
    from concourse import bass_utils

    m = np.asarray(m, dtype=np.float32)
    x = np.asarray(x, dtype=np.float32)
    nc = _build()
    res = bass_utils.run_bass_kernel_spmd(
        nc, _in_maps(m, x), core_ids=list(range(B)), trace=_trace
    )
    kernel.last_results = res
    y = np.stack(
        [np.asarray(res.results[b]["y"], dtype=np.float32) for b in range(B)],
        axis=0,
    )
    return y


# revision 35
# speedup vs baseline: 1.2643x; 1.0173x over previous
"""Trainium2 Bass kernel for nn_CCM: per-pixel complex 3x3 conv mask.

Math (per batch element b, sharded 1 batch element per NeuronCore):
  y[t,f] = sum_{n=0..8} A_n[t,f] * X[t+i(n)-2, f+j(n)-1]   (complex)
with A_n = m_n + w * m_{9+n} + conj(w) * m_{18+n}, w = -1/2 + i*sqrt(3)/2:
  Ar_n = m_n - 0.5*(m_{9+n} + m_{18+n})
  Ai_n = s * (m_{9+n} - m_{18+n}),  s = sqrt(3)/2
X = xr + i*xi, zero padded (causal in t: 2 top; symmetric in f: 1,1).

v4 design (trace-driven, from v1-v3):
- The kernel is jointly limited by the 29.8MB fp32 load stream (~84us at
  358GB/s) and the DVE. Everything else is arranged to keep both saturated:
- Per-tap products accumulate into PSUM fp32 via identity-weight matmuls on
  the PE (removes all accumulation adds from the DVE; better precision than
  bf16 accumulators).
- The +-s basis scale lives in two extra SCALED x planes (xrs=+s*xr,
  xin=-s*xi); th=-0.5*(m9+m18) runs on ACT with one-tap lookahead so the
  DVE never waits. DVE per tap: t1, d, ar + 4 plain 2x tensor_tensor muls.
- x stages via HWDGE (sync) as fp32 so the SWDGE m-stream starts ~4us
  earlier; x transposes run in fp32, plane copies cast to bf16.
- PSUM holds 4096 fp32/partition but the output needs 2*2056 so the f=256
  column accumulates in SBUF via tiny strided DVE adds (j==2 taps are zero
  there and skipped); it stores via a direct scatter SWDGE cast DMA.
- Output: per (f-half, comp) ONE psum bank collects all 8 tau-row
  transposes (start=False accumulate-into-disjoint-elements), then ONE
  contiguous-source copy into the bf16 staging tile; casting SWDGE stores.
- Tap 8's m tiles load in tau-quarters so its products/dr ains/transposes
  pipeline with the last DMA arrivals.
- PSUM banks are time-shared via same-tag tile reuse: head x-transposes ->
  accumulators -> tail output collectors.
"""

import sys
import numpy as np

sys.path.insert(0, "/opt/trn_rl_repo")

B = 8
C = 27
T = 1000
F = 257
TP = 125          # partitions
TAU = 8           # t = 8*p + tau
NS = 10           # slots in x planes: tau in [-2, 8)
SROW = 260        # x plane slot row width (elements)
MW = TAU * F      # 2056: m / prod tile width (flat, rows of 257)
AW = TAU * 256    # 2048: psum accumulator width (rows of 256)
PLW = NS * SROW   # 2600: x plane width
QW = 2 * F        # 514: one tau-quarter of a flat m plane
SQ3H = float(np.sqrt(3.0) / 2.0)

_CACHE = {}


def _emit(ctx, tc, m_ap, x_ap, id_ap, y_ap):
    import concourse.mybir as mybir

    nc = tc.nc
    f32 = mybir.dt.float32
    bf16 = mybir.dt.bfloat16
    FCS = [(0, 128), (128, 128), (256, 1)]   # f chunks for transposes
    SLOT_GROUPS = [(0, 4), (4, 4), (8, 2)]   # batches of slots per psum tile

    const = ctx.enter_context(tc.tile_pool(name="const", bufs=1))
    mcpool = ctx.enter_context(tc.tile_pool(name="mc", bufs=1))
    xpool = ctx.enter_context(tc.tile_pool(name="xstage", bufs=1))
    # One PSUM pool; the 8 banks rotate roles via tag reuse:
    # head ptg transposes -> accr/acci accumulators -> tail out collectors.
    psum = ctx.enter_context(tc.tile_pool(name="psum", bufs=1, space="PSUM"))
    BTAGS = [f"bank{i}" for i in range(8)]

    # ---- SWDGE casting-load order: tap 0's pair first (it gates the DVE's
    # first op), then identb (gates PE transposes), tap 0's single, then x,
    # then the remaining taps. ident f32 via HWDGE sync (slow startup fine:
    # only the tail's fp32 pair-transposes read it).
    mp = {}
    ms = {}

    def load_mpair(n):
        # two sequential per-plane DMAs: a single c-strided pair DMA whose
        # descriptors alternate between HBM regions 9 planes apart measurably
        # drops SWDGE throughput (DRAM page locality)
        p = mcpool.tile([TP, 2 * MW], bf16, tag=f"mp{n}", name=f"mp{n}")
        nc.gpsimd.dma_start(
            p[:, 0:MW], m_ap[9 + n].rearrange("(p t) f -> p (t f)", p=TP)
        )
        nc.gpsimd.dma_start(
            p[:, MW:2 * MW], m_ap[18 + n].rearrange("(p t) f -> p (t f)", p=TP)
        )
        mp[n] = p

    def load_msingle(n):
        s = mcpool.tile([TP, MW], bf16, tag=f"ms{n}", name=f"ms{n}")
        nc.gpsimd.dma_start(s[:], m_ap[n].rearrange("(p t) f -> p (t f)", p=TP))
        ms[n] = s

    # tap 0 loads in tau-halves so its prep/products start earlier
    mp0 = mcpool.tile([TP, 2 * MW], bf16, tag="mp0", name="mp0")
    ms0 = mcpool.tile([TP, MW], bf16, tag="ms0", name="ms0")
    HH = 4 * F
    for h in (0, 1):
        hs = slice(h * HH, (h + 1) * HH)
        nc.gpsimd.dma_start(
            mp0[:, 0:MW][:, hs],
            m_ap[9].rearrange("(p t) f -> p (t f)", p=TP)[:, hs])
        nc.gpsimd.dma_start(
            mp0[:, MW:2 * MW][:, hs],
            m_ap[18].rearrange("(p t) f -> p (t f)", p=TP)[:, hs])
        nc.gpsimd.dma_start(
            ms0[:, hs], m_ap[0].rearrange("(p t) f -> p (t f)", p=TP)[:, hs])
    mp[0], ms[0] = mp0, ms0

    identb = const.tile([128, 128], bf16, tag="identb")
    nc.gpsimd.dma_start(identb[:], id_ap)

    ident = const.tile([128, 128], f32, tag="ident")
    nc.sync.dma_start(ident[:], id_ap)

    xns = []
    for ci, (f0, fw) in enumerate(FCS):
        xn = xpool.tile([fw, (T + 2) * 2], bf16, tag=f"xn{f0}", name=f"xn{f0}")
        nc.vector.memset(xn[:, 0:4], 0.0)
        nc.gpsimd.dma_start(
            xn[:, 4:], x_ap[f0:f0 + fw].rearrange("f t c -> f (t c)")
        )
        xns.append(xn)

    for n in range(1, 9):
        load_mpair(n)
        load_msingle(n)

    # ---- x planes (bf16): f origin at col 1, zero pads at cols 0, 258, 259.
    # xr/xi are plain; xrs = +s*xr and xin = -s*xi carry the basis scale.
    planes = {}
    for nm in ("xr", "xi", "xrs", "xin"):
        p = const.tile([TP, PLW], bf16, tag=nm, name=nm)
        if nm in ("xr", "xi"):
            pv = p.rearrange("p (s w) -> p s w", w=SROW)
            nc.vector.memset(pv[:, :, 0:1], 0.0)
            nc.vector.memset(pv[:, :, 258:260], 0.0)
        planes[nm] = p

    # sideband (f=256) accumulators, one per complex component
    sbr = const.tile([TP, TAU], bf16, tag="sbr")
    sbi = const.tile([TP, TAU], bf16, tag="sbi")
    nc.vector.memset(sbr[:], 0.0)
    nc.vector.memset(sbi[:], 0.0)

    # prep for tap 0 sits ahead of the head copies in the DVE queue: its
    # ops wait only on the m0 DMAs, measuring/overlapping the copy phase.
    prep = ctx.enter_context(tc.tile_pool(name="prep", bufs=2))

    def prep_td(n, r0=0, rn=TAU, th_on_dve=False):
        sl = slice(r0 * F, (r0 + rn) * F)
        m9 = mp[n][:, 0:MW][:, sl]
        m18 = mp[n][:, MW:2 * MW][:, sl]
        t1 = prep.tile([TP, rn * F], bf16, tag="t1", name="t1", bufs=1)
        nc.vector.tensor_add(t1[:], m9, m18)
        d = prep.tile([TP, rn * F], bf16, tag="d", name="d", bufs=1)
        nc.vector.tensor_sub(d[:], m9, m18)
        th = prep.tile([TP, rn * F], bf16, tag="th", name="th")
        if th_on_dve:
            nc.vector.tensor_scalar_mul(th[:], t1[:], -0.5)
        else:
            nc.scalar.mul(th[:], t1[:], -0.5)
        return th, d

    def prep_ar(n, th, r0=0, rn=TAU):
        ar = prep.tile([TP, rn * F], bf16, tag="ar", name="ar", bufs=1)
        nc.vector.tensor_add(ar[:], th[:], ms[n][:, r0 * F:(r0 + rn) * F])
        return ar

    # tap-0 prep in tau-halves (all DVE; first ops in the queue): each
    # half only waits its own half-DMAs
    d0_e = prep.tile([TP, MW], bf16, tag="d", name="d0", bufs=1)
    ar0_e = prep.tile([TP, MW], bf16, tag="ar", name="ar0", bufs=1)
    t10 = prep.tile([TP, MW], bf16, tag="t1", name="t10", bufs=1)
    th0 = prep.tile([TP, MW], bf16, tag="th", name="th0")
    for h in (0, 1):
        hs = slice(h * 4 * F, (h + 1) * 4 * F)
        m9h = mp0[:, 0:MW][:, hs]
        m18h = mp0[:, MW:2 * MW][:, hs]
        nc.vector.tensor_add(t10[:, hs], m9h, m18h)
        nc.vector.tensor_sub(d0_e[:, hs], m9h, m18h)
        nc.vector.tensor_scalar_mul(th0[:, hs], t10[:, hs], -0.5)
        nc.vector.tensor_add(ar0_e[:, hs], th0[:, hs], ms0[:, hs])

    # ---- transpose x into the planes (fp32 PE transposes into psum banks).
    # Copies cast fp32->bf16: slot groups g0 on the idle DVE, g1/g2 on ACT.
    # Scaled planes built per group right after (DVE for g0, ACT for rest).
    bank_rr = 0
    for gi, (g0, gn) in enumerate(SLOT_GROUPS):
        for ci, (f0, fw) in enumerate(FCS):
            xn3 = xns[ci].rearrange("f (t c) -> f t c", c=2)
            for q, nm in enumerate(("xr", "xi")):
                pA = planes[nm].rearrange("p (s w) -> p s w", w=SROW)
                ptg = psum.tile(
                    [TP, 512], bf16, tag=BTAGS[bank_rr % 8], name="ptg",
                    padded_shape=[128, 1024],
                )
                bank_rr += 1
                for u in range(gn):
                    ts = g0 + u
                    nc.tensor.transpose(
                        ptg[0:TP, 128 * u:128 * u + fw],
                        xn3[0:fw, ts:ts + TAU * (TP - 1) + 1:TAU, q],
                        identb[0:fw, 0:fw],
                    )
                src = ptg.rearrange("p (u w) -> p u w", w=128)[0:TP, 0:gn, 0:fw]
                dst = pA[:, g0:g0 + gn, 1 + f0:1 + f0 + fw]
                if gi == 0:
                    nc.vector.tensor_copy(dst, src)
                else:
                    nc.scalar.copy(dst, src)
        # scaled planes for this slot group
        for src_nm, dst_nm, sc in (("xr", "xrs", SQ3H), ("xi", "xin", -SQ3H)):
            sv = planes[src_nm].rearrange("p (s w) -> p s w", w=SROW)
            dv = planes[dst_nm].rearrange("p (s w) -> p s w", w=SROW)
            if gi == 0:
                nc.vector.tensor_scalar_mul(
                    dv[:, g0:g0 + gn, :], sv[:, g0:g0 + gn, :], sc)
            else:
                nc.scalar.mul(dv[:, g0:g0 + gn, :], sv[:, g0:g0 + gn, :], sc)

    prod = ctx.enter_context(tc.tile_pool(name="prod", bufs=4))
    yop = ctx.enter_context(tc.tile_pool(name="yop", bufs=1))

    # ---- PSUM accumulators: accr rows of 256 in banks 0-3, acci in 4-7.
    # 512 fp32 = one bank = two tau rows.
    accr_c = [
        psum.tile([TP, 512], f32, tag=BTAGS[c], name=f"accr{c}",
                  padded_shape=[128, 512])
        for c in range(4)
    ]
    acci_c = [
        psum.tile([TP, 512], f32, tag=BTAGS[4 + c], name=f"acci{c}",
                  padded_shape=[128, 512])
        for c in range(4)
    ]

    idw = identb[0:TP, 0:TP]

    def products(n, ar, d, r0, rn):
        """4 plain-mul product tiles for tau rows [r0, r0+rn) (tile-local)."""
        i, j = divmod(n, 3)

        def xv(nm):
            return planes[nm].rearrange("p (s w) -> p s w", w=SROW)[
                :, i + r0:i + r0 + rn, j:j + F]

        a8 = ar.rearrange("p (r w) -> p r w", w=F)
        d8 = d.rearrange("p (r w) -> p r w", w=F)
        w = rn * F
        p0 = prod.tile([TP, w], bf16, tag="P", name="p0")
        p1 = prod.tile([TP, w], bf16, tag="P", name="p1")
        p2 = prod.tile([TP, w], bf16, tag="P", name="p2")
        p3 = prod.tile([TP, w], bf16, tag="P", name="p3")
        nc.vector.tensor_mul(p0.rearrange("p (r w) -> p r w", w=F), a8, xv("xr"))
        nc.vector.tensor_mul(p1.rearrange("p (r w) -> p r w", w=F), a8, xv("xi"))
        nc.vector.tensor_mul(p2.rearrange("p (r w) -> p r w", w=F), d8, xv("xin"))
        nc.vector.tensor_mul(p3.rearrange("p (r w) -> p r w", w=F), d8, xv("xrs"))
        return p0, p1, p2, p3

    def accum_main(n, tiles, r0, rn, last):
        """PE-accumulate tau rows [r0, r0+rn) of the products into PSUM."""
        p0, p1, p2, p3 = tiles
        first = n == 0
        for c in range(r0 // 2, (r0 + rn) // 2):
            lo = c * 2 - r0
            for acc, pa, pb in ((accr_c[c], p0, p2), (acci_c[c], p1, p3)):
                for k, pt in enumerate((pa, pb)):
                    pv = pt.rearrange("p (r w) -> p r w", w=F)[
                        :, lo:lo + 2, 0:256]
                    nc.tensor.matmul(
                        acc[:], idw, pv,
                        start=(first and k == 0),
                        stop=(last and k == 1),
                    )

    def accum_sb(n, tiles, r0, rn):
        if n % 3 == 2:
            return
        p0, p1, p2, p3 = tiles
        for acc, pa, pb in ((sbr, p0, p2), (sbi, p1, p3)):
            for pt in (pa, pb):
                pv = pt.rearrange("p (r w) -> p r w", w=F)[:, 0:rn, 256]
                nc.vector.tensor_add(acc[:, r0:r0 + rn], acc[:, r0:r0 + rn], pv)

    # drained accumulators in (f-major, tau-minor) bf16 layout: adjacent tau
    # pairs (t=8p+2q, +1) are then adjacent bytes, so the output transposes
    # can run on fp32-reinterpreted PAIRS (psum matmul writes need 4B align).
    acc_s = [
        const.tile([TP, AW], bf16, tag="accr_s", name="accr_s"),
        const.tile([TP, AW], bf16, tag="acci_s", name="acci_s"),
    ]
    acc32 = [a.bitcast(f32) for a in acc_s]

    yo01 = yop.tile([128, 2 * T * 2], f32, tag="yo01", name="yo01")
    yviews = [
        yo01[:, 0:T * 2].rearrange("f (t c) -> f t c", c=2),
        yo01[:, T * 2:].rearrange("f (t c) -> f t c", c=2),
    ]

    # output collector psum banks: one per (f-half, comp); the 4 fp32 pair
    # transposes land strided (pair position = 4p+q fp32) into one bank,
    # leaving it t-contiguous bf16; then ONE copy into yo01.
    # Banks are reused in drain-retirement order (quarter c frees c and 4+c).
    OBANK = {(0, 0): 0, (0, 1): 4, (1, 0): 1, (1, 1): 5}
    obank = {}

    def drain_chunk(c):
        for comp, acc in ((0, accr_c[c]), (1, acci_c[c])):
            src = acc.rearrange("p (r f) -> p f r", r=2)
            dst = acc_s[comp].rearrange("p (f r) -> p f r", r=TAU)[
                :, :, 2 * c:2 * c + 2]
            nc.scalar.copy(dst, src)

    out_done = {k: 0 for k in OBANK}

    def out_rows(q):
        """After drain_chunk(q): transpose every tau PAIR that is both
        drained (pair <= q) and whose collector bank is retired (f0's
        banks 0/4 retire at q>=0, f1's banks 1/5 at q>=1)."""
        for ci in (0, 1):
            if q < ci:
                continue
            f0, fw = FCS[ci]
            for comp in (0, 1):
                key = (ci, comp)
                if key not in obank:
                    obank[key] = psum.tile(
                        [128, T // 2], f32, tag=BTAGS[OBANK[key]],
                        name=f"ob{ci}{comp}", padded_shape=[128, 512],
                    )
                ob = obank[key]
                a32 = acc32[comp].rearrange("p (f r) -> p f r", r=TAU // 2)
                for rp in range(out_done[key], q + 1):
                    # each strided transpose is its own single-matmul group
                    # (start=False accumulation into untouched elements is
                    # not safe; disjoint start=True writes are).
                    nc.tensor.matmul(
                        ob[0:fw, rp:rp + 4 * (TP - 1) + 1:4],
                        a32[:, f0:f0 + fw, rp],
                        ident[0:TP, 0:TP],
                        is_transpose=True,
                    )
                out_done[key] = q + 1

    for n in range(C // 3):
        if n == 0:
            ar0, dd = ar0_e, d0_e
        if n < 8:
            nxt_td = prep_td(n + 1)
            tiles = products(n, ar0, dd, 0, TAU)
            accum_main(n, tiles, 0, TAU, last=False)
            accum_sb(n, tiles, 0, TAU)
            ar0, dd = prep_ar(n + 1, nxt_td[0]), nxt_td[1]
        else:
            # tap 8 in tau-halves: the PE accumulate chain of half 0 runs
            # under half 1's products, shortening the raw tail
            for h in (0, 1):
                hs = slice(h * 4 * F, (h + 1) * 4 * F)
                tiles = products(n, ar0[:, hs], dd[:, hs], 4 * h, 4)
                accum_main(n, tiles, 4 * h, 4, last=True)
                accum_sb(n, tiles, 4 * h, 4)
            # drains + output transposes staged per retired psum chunk
            for q in range(4):
                drain_chunk(q)
                out_rows(q)

    # ---- one contiguous-psum-source copy per (f-half, comp), then stores.
    # DVE takes the real comps, ACT the imag; casting SWDGE stores per half.
    for ci in (0, 1):
        for comp in (0, 1):
            dst = yviews[ci][0:128, :, comp]
            src = obank[(ci, comp)].bitcast(bf16)[0:128, 0:T]
            if comp == 0:
                nc.vector.tensor_copy(dst, src)
            else:
                nc.scalar.copy(dst, src)
        nc.gpsimd.dma_start(
            y_ap[128 * ci:128 * (ci + 1)].rearrange("f t c -> f (t c)"),
            yo01[:, T * 2 * ci:T * 2 * (ci + 1)],
        )
    # f=256 sideband: interleave (t, c) as f32 in SBUF (2 tiny ACT copies),
    # then one contiguous-row store (64B runs per partition).
    sbri = yop.tile([TP, 2 * TAU], f32, tag="sbri", name="sbri")
    sbv = sbri.rearrange("p (t c) -> p t c", c=2)
    nc.scalar.copy(sbv[:, :, 0], sbr[:])
    nc.scalar.copy(sbv[:, :, 1], sbi[:])
    nc.gpsimd.dma_start(
        y_ap[256].rearrange("(p t) c -> p (t c)", p=TP), sbri[:]
    )


def _build():
    if "nc" in _CACHE:
        return _CACHE["nc"]
    from contextlib import ExitStack
    from concourse import bacc, mybir
    import concourse.tile as tile

    f32 = mybir.dt.float32
    nc = bacc.Bacc("TRN2", target_bir_lowering=False, debug=False, num_devices=B)
    m_d = nc.dram_tensor("m", (C, T, F), f32, kind="ExternalInput")
    x_d = nc.dram_tensor("x", (F, T, 2), f32, kind="ExternalInput")
    id_d = nc.dram_tensor("ident", (128, 128), f32, kind="ExternalInput")
    y_d = nc.dram_tensor("y", (F, T, 2), f32, kind="ExternalOutput")

    with tile.TileContext(nc) as tc:
        with ExitStack() as ctx:
            _emit(ctx, tc, m_d.ap(), x_d.ap(), id_d.ap(), y_d.ap())
    nc.compile()
    _CACHE["nc"] = nc
    return nc


def _in_maps(m, x):
    ident = np.eye(128, dtype=np.float32)
    return [
        {"m": np.ascontiguousarray(m[b]), "x": np.ascontiguousarray(x[b]),
         "ident": ident}
        for b in range(B)
    ]


def kernel(m, x, v, _trace=False):
    from concourse import bass_utils

    m = np.asarray(m, dtype=np.float32)
    x = np.asarray(x, dtype=np.float32)
    nc = _build()
    res = bass_utils.run_bass_kernel_spmd(
        nc, _in_maps(m, x), core_ids=list(range(B)), trace=_trace
    )
    kernel.last_results = res
    y = np.stack(
        [np.asarray(res.results[b]["y"], dtype=np.float32) for b in range(B)],
        axis=0,
    )
    return y
